# revision 9
# baseline (speedup 1.0000x reference)
"""CP-decomposed 3x3 conv on 8 TRN2 NeuronCores.

Math: out[f,i,j] = sum_{h,w,c,r} in[c,i+h,j+w] * f1[h,r] * f2[w,r] * f3[c,r] * f0[f,r]

Factorization used on-device (per core, over its 32 output rows):
  stage A: t2[r, n]  = sum_h sum_c (f3[c,r]*f1[h,r]) * x[c, n + h*W]     (3 matmuls, K=C)
  stage B: out[f, n] = sum_w sum_r (f2[w,r]*f0[f,r]) * t2[r, n + w]      (3 matmuls, K=R)
where n flattens (row, col) with row pitch W=256; output cols 254/255 of each
row are garbage and are dropped at host gather.

Per-core layout (v4): the 32 output rows split into two 16-row halves. SBUF
partitions 0-63 hold half0's input rows [0,18), partitions 64-127 hold half1's
rows [16,34). The host ships x with the 2 halo rows duplicated ([C, 36, W]:
rows 0-17 then 16-33) so a single rearranged DMA covers all 128 partitions.

The whole input lives in one SBUF tile [128, 4608], loaded in 4 column-chunk
DMAs ordered by when compute needs them: sync ring carries cols [0,1536) and
[1536,2560); scalar carries the weights then cols [2560,3584) and [3584,4608).
Range-based Tile deps let stage A's first matmuls start after the first chunk.

Per quad-iter q (4 output rows per half): stage A runs 12 matmuls (3 taps x 4
PE quadrants via tile_position auto-derived from lhsT/psum base partitions)
into p1q [128,1024]; the t2 evacuation is split across vector+scalar so both
engines carry half. Stage B runs per half: 6 matmuls (3 taps x 2 row-group
chunks, alternating row groups so fill/drain pipelines) into p2q [128,1024],
evacuated [128,1024] f32->bf16 on alternating engines, each evac immediately
followed by its 4-row y DMA on the ring matching the evac engine (vector->
sync, scalar->scalar) so a waiting DMA never head-blocks the copy engine.

I/O is bf16 both ways; output rows are written 256-wide, 4 rows per DMA, and
trimmed to 254 at host gather.

Sharding: output rows (Ho=254) split across 8 cores: cores 0-6 get rows
[32i, 32i+32); core 7 processes rows [222, 254) via a shifted window (its
first 2 rows duplicate core 6's tail and are dropped at gather).
"""

import sys

sys.path.insert(0, "/opt/trn_rl_repo")

import numpy as np

# Problem constants (hardcoded per contract)
C = 64
H = 256
W = 256
FH = 3
FW = 3
RANK = 64
F = 128
HO = H - FH + 1  # 254
WO = W - FW + 1  # 254
NCORES = 8
ROWS = 32  # output rows per core
IN_ROWS = ROWS + 2  # 34
HALF_OUT = ROWS // 2  # 16 output rows per half
HALF_IN = HALF_OUT + 2  # 18 input rows per half
HCOLS = HALF_IN * W  # 4608 input cols per half
CHUNK = 512  # output elements per chunk (= 2 rows x 256)
NQUAD = 4  # quad-iters; each covers 2 chunks per half (4 rows per half)

COMPUTE_DT = "bf16"
# Ablation switches for benchmarking: subset of
# {"in_dma", "out_dma", "stage_a", "stage_b", "copies"}
ABLATE = set()

_PROGRAM_CACHE = {}


def _np_compute_dtype():
    import ml_dtypes

    if COMPUTE_DT == "fp16":
        return np.dtype(ml_dtypes.float16)
    return np.dtype(ml_dtypes.bfloat16)


def build_program(
    rows=ROWS,
    compute_dt=None,
    num_devices=NCORES,
    reps=1,
    paired=None,  # unused; kept for bench.py compat
    bench_internal=False,
):
    """Build + compile the per-core Bass program."""
    from concourse import bacc, mybir, tile

    compute_dt = compute_dt or COMPUTE_DT
    dt_c = mybir.dt.float16 if compute_dt == "fp16" else mybir.dt.bfloat16
    dt_f32 = mybir.dt.float32

    assert rows == ROWS

    nc = bacc.Bacc(
        "TRN2", target_bir_lowering=False, debug=False, num_devices=num_devices
    )
    if bench_internal:
        x = nc.dram_tensor("x_int", [2 * C, HCOLS], dt_c).ap()
        wa2 = nc.dram_tensor("wa2_int", [2 * C, FH * RANK], dt_c).ap()
        wb2 = nc.dram_tensor("wb2_int", [2 * RANK, FW * F], dt_c).ap()
        y = nc.dram_tensor("y_int", [F, ROWS, W], dt_c).ap()
        tin = nc.dram_tensor("tin", [1, 16], dt_f32, kind="ExternalInput").ap()
        tout = nc.dram_tensor("tout", [1, 16], dt_f32, kind="ExternalOutput").ap()
    else:
        x = nc.dram_tensor("x", [2 * C, HCOLS], dt_c, kind="ExternalInput").ap()
        wa2 = nc.dram_tensor("wa2", [2 * C, FH * RANK], dt_c, kind="ExternalInput").ap()
        wb2 = nc.dram_tensor("wb2", [2 * RANK, FW * F], dt_c, kind="ExternalInput").ap()
        y = nc.dram_tensor("y", [F, ROWS, W], dt_c, kind="ExternalOutput").ap()

    with tile.TileContext(nc) as tc:
        with (
            tc.tile_pool(name="xin", bufs=2) as xin_pool,
            tc.tile_pool(name="wgt", bufs=2) as wgt_pool,
            tc.tile_pool(name="t2", bufs=3) as t2_pool,
            tc.tile_pool(name="ot", bufs=2) as ot_pool,
            tc.tile_pool(name="p1", bufs=2, space="PSUM") as p1_pool,
            tc.tile_pool(name="p2", bufs=2, space="PSUM") as p2_pool,
        ):

            def body():
                # (g c) partition layout: partitions 0-63 = half0 rows 0-17,
                # 64-127 = half1 rows 16-33 (host duplicates the halo rows).
                X = xin_pool.tile([2 * C, HCOLS], dt_c, tag="x")
                WA = wgt_pool.tile([2 * C, FH * RANK], dt_c, tag="wa")
                WB = wgt_pool.tile([2 * RANK, FW * F], dt_c, tag="wb")
                # weights via SWDGE: off both HWDGE rings, land ~1.3us
                nc.gpsimd.dma_start(out=WA[:], in_=wa2[:])
                nc.gpsimd.dma_start(out=WB[:], in_=wb2[:])
                if "in_dma" in ABLATE:
                    nc.vector.memset(X[:, 0:8], 0.0)
                else:
                    # all reads on sync (reads cap ~215 GB/s aggregate, ring
                    # split buys nothing) ordered by need; scalar ring stays
                    # free so the act-table load never delays an input chunk
                    nc.sync.dma_start(out=X[:, 0:1024], in_=x[:, 0:1024])
                    nc.sync.dma_start(out=X[:, 1024:1536], in_=x[:, 1024:1536])
                    nc.sync.dma_start(out=X[:, 1536:2560], in_=x[:, 1536:2560])
                    nc.sync.dma_start(out=X[:, 2560:3584], in_=x[:, 2560:3584])
                    nc.sync.dma_start(out=X[:, 3584:4608], in_=x[:, 3584:4608])

                def stage_a(q):
                    # psum slots (pa, col ca): pa = chunk parity, ca = half.
                    # Quadrant (64*half, pa): all four distinct -> 4-way.
                    p1q = p1_pool.tile([2 * C, 2 * CHUNK], dt_f32)
                    if "stage_a" in ABLATE:
                        nc.vector.memset(p1q[:, 0:8], 0.0)
                    else:
                        if q == 0:
                            # parity-phased: par0 chains need only x cols
                            # [0,1024) (first DMA chunk), par1 cols [512,1536)
                            order = [
                                (h, half, pa)
                                for pa in (0, 64)
                                for h in range(FH)
                                for half in (0, 1)
                            ]
                        else:
                            order = [
                                (h, half, pa)
                                for h in range(FH)
                                for half, pa in ((0, 0), (0, 64), (1, 0), (1, 64))
                            ]
                        for h, half, pa in order:
                            if True:
                                l = 2 * q + (1 if pa else 0)
                                base = l * CHUNK + h * W
                                nc.tensor.matmul(
                                    out=p1q[
                                        pa : pa + RANK,
                                        half * CHUNK : (half + 1) * CHUNK,
                                    ],
                                    lhsT=WA[
                                        half * C : (half + 1) * C,
                                        h * RANK : (h + 1) * RANK,
                                    ],
                                    rhs=X[
                                        half * C : (half + 1) * C,
                                        base : base + CHUNK,
                                    ],
                                    start=(h == 0),
                                    stop=(h == FH - 1),
                                    skip_group_check=True,
                                )
                    t2q = t2_pool.tile([2 * RANK, 2 * CHUNK + 4], dt_c, tag="t2")
                    # pad cols feed only discarded output columns; cheap DVE
                    # memset (gpsimd's Q7 dispatch latency would gate stage B)
                    nc.vector.memset(t2q[:, 2 * CHUNK : 2 * CHUNK + 4], 0.0)
                    if "copies" in ABLATE:
                        nc.vector.memset(t2q[:, 0:8], 0.0)
                    else:
                        # split the evac across both engines so neither
                        # becomes the A->B critical path
                        nc.vector.tensor_copy(
                            out=t2q[:, 0:CHUNK], in_=p1q[:, 0:CHUNK]
                        )
                        nc.scalar.copy(
                            out=t2q[:, CHUNK : 2 * CHUNK],
                            in_=p1q[:, CHUNK : 2 * CHUNK],
                        )
                    return t2q

                yf = y.rearrange("f r w -> f (r w)")

                def stage_b(q, t2q):
                    # t2q[pa:pa+64, half*512+w : ...] holds chunk (half, 2q+pa/64)
                    # -> p2q cols (pa/64)*512 -> y rows half*16+4q..+4.
                    for half in range(2):
                        p2q = p2_pool.tile([F, 2 * CHUNK], dt_f32, tag="p2")
                        if "stage_b" in ABLATE:
                            nc.vector.memset(p2q[:, 0:8], 0.0)
                        else:
                            for w in range(FW):
                                for pa in (0, 64):
                                    nc.tensor.matmul(
                                        out=p2q[
                                            :,
                                            (pa // 64) * CHUNK : (pa // 64 + 1)
                                            * CHUNK,
                                        ],
                                        lhsT=WB[
                                            pa : pa + RANK, w * F : (w + 1) * F
                                        ],
                                        rhs=t2q[
                                            pa : pa + RANK,
                                            half * CHUNK
                                            + w : (half + 1) * CHUNK
                                            + w,
                                        ],
                                        start=(w == 0),
                                        stop=(w == FW - 1),
                                        skip_group_check=True,
                                    )
                        if "out_dma" in ABLATE:
                            nc.vector.memset(p2q[:, 8:16], 1.0)
                            continue
                        ot = ot_pool.tile([F, 2 * CHUNK], dt_c, tag=f"ot{half}")
                        r0 = half * HALF_OUT + 4 * q
                        if q == NQUAD - 1:
                            # final quad: split evac across both engines and
                            # store 2-row pieces -> short tail
                            nc.vector.tensor_copy(
                                out=ot[:, 0:CHUNK], in_=p2q[:, 0:CHUNK]
                            )
                            nc.scalar.copy(
                                out=ot[:, CHUNK : 2 * CHUNK],
                                in_=p2q[:, CHUNK : 2 * CHUNK],
                            )
                            nc.sync.dma_start(
                                out=yf[:, r0 * W : (r0 + 2) * W],
                                in_=ot[:, 0:CHUNK],
                            )
                            nc.sync.dma_start(
                                out=yf[:, (r0 + 2) * W : (r0 + 4) * W],
                                in_=ot[:, CHUNK : 2 * CHUNK],
                            )
                            continue
                        if (q + half) % 2 == 0:
                            nc.vector.tensor_copy(out=ot[:], in_=p2q[:])
                        else:
                            nc.scalar.copy(out=ot[:], in_=p2q[:])
                        # all stores issue from the sync ring (idle after the
                        # reads) so DMA issue never holds a copy engine's SEQ
                        nc.sync.dma_start(
                            out=yf[:, r0 * W : (r0 + 4) * W], in_=ot[:]
                        )

                pending = None
                for q in range(NQUAD + 1):
                    t2q = stage_a(q) if q < NQUAD else None
                    if pending is not None:
                        stage_b(pending[0], pending[1])
                    pending = (q, t2q) if t2q is not None else None

            if reps == 1:
                body()
            else:
                with tc.For_i(0, reps, 1):
                    body()
            if bench_internal:
                nc.sync.dma_start(out=tout[:], in_=tin[:])

    nc.compile()
    return nc


def _get_program():
    key = (ROWS, COMPUTE_DT)
    if key not in _PROGRAM_CACHE:
        _PROGRAM_CACHE[key] = build_program()
    return _PROGRAM_CACHE[key]


def make_weight_inputs(factor0, factor1, factor2, factor3, np_dt=None):
    np_dt = np_dt or _np_compute_dtype()
    f0 = np.asarray(factor0, np.float32)
    f1 = np.asarray(factor1, np.float32)
    f2 = np.asarray(factor2, np.float32)
    f3 = np.asarray(factor3, np.float32)
    # wa[c, h*RANK+r] = f3[c,r] * f1[h,r], duplicated into both halves
    wa = (f3[:, None, :] * f1[None, :, :]).reshape(C, FH * RANK)
    wa2 = np.concatenate([wa, wa], axis=0).astype(np_dt)
    # wb[r, w*F+f] = f2[w,r] * f0[f,r], duplicated into both halves
    wb = (f2.T[:, :, None] * f0.T[:, None, :]).reshape(RANK, FW * F)
    wb2 = np.concatenate([wb, wb], axis=0).astype(np_dt)
    return np.ascontiguousarray(wa2), np.ascontiguousarray(wb2)


ROW_STARTS = [0, 32, 64, 96, 128, 160, 192, 222]


def kernel(input, factor0, factor1, factor2, factor3):
    from concourse.bass_utils import run_bass_kernel_spmd

    nc = _get_program()
    np_dt = _np_compute_dtype()
    wa2, wb2 = make_weight_inputs(factor0, factor1, factor2, factor3, np_dt)
    inp = np.asarray(input, np.float32).astype(np_dt)
    in_maps = []
    for s in ROW_STARTS:
        xs = inp[:, s : s + IN_ROWS, :]
        # partitions (g c): half0 rows 0-17, half1 rows 16-33 -> [2C, 18*W]
        xd = np.stack(
            [xs[:, 0:HALF_IN, :], xs[:, HALF_OUT:IN_ROWS, :]], axis=0
        ).reshape(2 * C, HCOLS)
        in_maps.append(
            {"x": np.ascontiguousarray(xd), "wa2": wa2, "wb2": wb2}
        )
    res = run_bass_kernel_spmd(nc, in_maps, list(range(NCORES))).results
    out = np.empty((F, HO, WO), np.float32)
    for i, s in enumerate(ROW_STARTS):
        ys = res[i]["y"][:, :, 0:WO].astype(np.float32)
        if i < NCORES - 1:
            out[:, s : s + ROWS, :] = ys
        else:
            out[:, 224:HO, :] = ys[:, 2:ROWS, :]
    return out


# revision 19
# speedup vs baseline: 1.1032x; 1.1032x over previous
"""CP-decomposed 3x3 conv on 8 TRN2 NeuronCores.

Math: out[f,i,j] = sum_{h,w,c,r} in[c,i+h,j+w] * f1[h,r] * f2[w,r] * f3[c,r] * f0[f,r]

Factorization used on-device (per core, over its 32 output rows):
  stage A: t2[r, n]  = sum_h sum_c (f3[c,r]*f1[h,r]) * x[c, n + h*W]     (3 matmuls, K=C)
  stage B: out[f, n] = sum_w sum_r (f2[w,r]*f0[f,r]) * t2[r, n + w]      (3 matmuls, K=R)
where n flattens (row, col) with row pitch W=256; output cols 254/255 of each
row are garbage and are dropped at host gather.

Per-core layout (v4): the 32 output rows split into two 16-row halves. SBUF
partitions 0-63 hold half0's input rows [0,18), partitions 64-127 hold half1's
rows [16,34). The host ships x with the 2 halo rows duplicated ([C, 36, W]:
rows 0-17 then 16-33) so a single rearranged DMA covers all 128 partitions.

The whole input lives in one SBUF tile [128, 4608], loaded in 4 column-chunk
DMAs ordered by when compute needs them: sync ring carries cols [0,1536) and
[1536,2560); scalar carries the weights then cols [2560,3584) and [3584,4608).
Range-based Tile deps let stage A's first matmuls start after the first chunk.

Per quad-iter q (4 output rows per half): stage A runs 12 matmuls (3 taps x 4
PE quadrants via tile_position auto-derived from lhsT/psum base partitions)
into p1q [128,1024]; the t2 evacuation is split across vector+scalar so both
engines carry half. Stage B runs per half: 6 matmuls (3 taps x 2 row-group
chunks, alternating row groups so fill/drain pipelines) into p2q [128,1024],
evacuated [128,1024] f32->bf16 on alternating engines, each evac immediately
followed by its 4-row y DMA on the ring matching the evac engine (vector->
sync, scalar->scalar) so a waiting DMA never head-blocks the copy engine.

I/O is bf16 both ways; output rows are written 256-wide, 4 rows per DMA, and
trimmed to 254 at host gather.

Sharding: output rows (Ho=254) split across 8 cores: cores 0-6 get rows
[32i, 32i+32); core 7 processes rows [222, 254) via a shifted window (its
first 2 rows duplicate core 6's tail and are dropped at gather).
"""

import sys

sys.path.insert(0, "/opt/trn_rl_repo")

import numpy as np

# Problem constants (hardcoded per contract)
C = 64
H = 256
W = 256
FH = 3
FW = 3
RANK = 64
F = 128
HO = H - FH + 1  # 254
WO = W - FW + 1  # 254
NCORES = 8
ROWS = 32  # output rows per core
IN_ROWS = ROWS + 2  # 34
HALF_OUT = ROWS // 2  # 16 output rows per half
HALF_IN = HALF_OUT + 2  # 18 input rows per half
HCOLS = HALF_IN * W  # 4608 input cols per half
CHUNK = 512  # output elements per chunk (= 2 rows x 256)
NQUAD = 4  # quad-iters; each covers 2 chunks per half (4 rows per half)

COMPUTE_DT = "bf16"
# Ablation switches for benchmarking: subset of
# {"in_dma", "out_dma", "stage_a", "stage_b", "copies"}
ABLATE = set()
# feature flags for A/B benching (t2buf3 = deeper t2 staging rotation,
# the only variant that measured faster; see session notes)
FEATURES = {"t2buf3"}

_PROGRAM_CACHE = {}


def _np_compute_dtype():
    import ml_dtypes

    if COMPUTE_DT == "fp16":
        return np.dtype(ml_dtypes.float16)
    return np.dtype(ml_dtypes.bfloat16)


def build_program(
    rows=ROWS,
    compute_dt=None,
    num_devices=NCORES,
    reps=1,
    paired=None,  # unused; kept for bench.py compat
    bench_internal=False,
    nquad=NQUAD,  # bench-only: fewer quad-iters to measure pipeline scaling
):
    """Build + compile the per-core Bass program."""
    from concourse import bacc, mybir, tile

    compute_dt = compute_dt or COMPUTE_DT
    dt_c = mybir.dt.float16 if compute_dt == "fp16" else mybir.dt.bfloat16
    dt_f32 = mybir.dt.float32

    assert rows == ROWS

    nc = bacc.Bacc(
        "TRN2", target_bir_lowering=False, debug=False, num_devices=num_devices
    )
    if bench_internal:
        x = nc.dram_tensor("x_int", [2 * C, HCOLS], dt_c).ap()
        wab = nc.dram_tensor("wab_int", [2 * C, FH * RANK + FW * F], dt_c).ap()
        wa2 = nc.dram_tensor("wa2_int", [2 * C, FH * RANK], dt_c).ap()
        wb2 = nc.dram_tensor("wb2_int", [2 * RANK, FW * F], dt_c).ap()
        y = nc.dram_tensor("y_int", [F, ROWS, W], dt_c).ap()
        tin = nc.dram_tensor("tin", [1, 16], dt_f32, kind="ExternalInput").ap()
        tout = nc.dram_tensor("tout", [1, 16], dt_f32, kind="ExternalOutput").ap()
    else:
        x = nc.dram_tensor("x", [2 * C, HCOLS], dt_c, kind="ExternalInput").ap()
        if "wab" in FEATURES:
            wab = nc.dram_tensor(
                "wab", [2 * C, FH * RANK + FW * F], dt_c, kind="ExternalInput"
            ).ap()
        else:
            wa2 = nc.dram_tensor("wa2", [2 * C, FH * RANK], dt_c, kind="ExternalInput").ap()
            wb2 = nc.dram_tensor("wb2", [2 * RANK, FW * F], dt_c, kind="ExternalInput").ap()
        y = nc.dram_tensor("y", [F, ROWS, W], dt_c, kind="ExternalOutput").ap()

    with tile.TileContext(nc) as tc:
        with (
            tc.tile_pool(name="xin", bufs=2) as xin_pool,
            tc.tile_pool(name="wgt", bufs=2) as wgt_pool,
            tc.tile_pool(name="t2", bufs=(3 if "t2buf3" in FEATURES else 2)) as t2_pool,
            tc.tile_pool(name="ot", bufs=2) as ot_pool,
            tc.tile_pool(
                name="p1",
                bufs=(1 if "p2deep" in FEATURES else 2),
                space="PSUM",
            ) as p1_pool,
            tc.tile_pool(
                name="p2",
                bufs=(3 if "p2deep" in FEATURES else 2),
                space="PSUM",
            ) as p2_pool,
        ):

            def body():
                # (g c) partition layout: partitions 0-63 = half0 rows 0-17,
                # 64-127 = half1 rows 16-33 (host duplicates the halo rows).
                X = xin_pool.tile([2 * C, HCOLS], dt_c, tag="x")
                if "wab" in FEATURES:
                    # both weights packed into one DMA on the scalar ring
                    WAB = wgt_pool.tile(
                        [2 * C, FH * RANK + FW * F], dt_c, tag="wab"
                    )
                    nc.scalar.dma_start(out=WAB[:], in_=wab[:])
                    WA = WAB[:, 0 : FH * RANK]
                    WB = WAB[:, FH * RANK : FH * RANK + FW * F]
                else:
                    WA = wgt_pool.tile([2 * C, FH * RANK], dt_c, tag="wa")
                    WB = wgt_pool.tile([2 * RANK, FW * F], dt_c, tag="wb")
                    nc.scalar.dma_start(out=WA[:], in_=wa2[:])
                    nc.scalar.dma_start(out=WB[:], in_=wb2[:])
                if "in_dma" in ABLATE:
                    nc.vector.memset(X[:, 0:8], 0.0)
                else:
                    # ordered by need: q0/q1 cols on sync, q2/q3 on scalar
                    xmax = nquad * 1024 + 512
                    nc.sync.dma_start(out=X[:, 0:1536], in_=x[:, 0:1536])
                    if xmax > 1536:
                        nc.sync.dma_start(
                            out=X[:, 1536:min(xmax, 2560)],
                            in_=x[:, 1536:min(xmax, 2560)],
                        )
                    if xmax > 2560:
                        nc.scalar.dma_start(
                            out=X[:, 2560:min(xmax, 3584)],
                            in_=x[:, 2560:min(xmax, 3584)],
                        )
                    if xmax > 3584:
                        nc.scalar.dma_start(
                            out=X[:, 3584:4608], in_=x[:, 3584:4608]
                        )

                def stage_a(q):
                    # psum slots (pa, col ca): pa = chunk parity, ca = half.
                    # Quadrant (64*half, pa): all four distinct -> 4-way.
                    p1q = p1_pool.tile([2 * C, 2 * CHUNK], dt_f32)
                    if "stage_a" in ABLATE:
                        nc.vector.memset(p1q[:, 0:8], 0.0)
                    else:
                        if q == 0 and "q0phase" in FEATURES:
                            # parity-phased: par0 chains need only x cols
                            # [0,1024) (first DMA chunk), par1 cols [512,1536)
                            order = [
                                (h, half, pa)
                                for pa in (0, 64)
                                for h in range(FH)
                                for half in (0, 1)
                            ]
                        else:
                            order = [
                                (h, half, pa)
                                for h in range(FH)
                                for half, pa in ((0, 0), (0, 64), (1, 0), (1, 64))
                            ]
                        for h, half, pa in order:
                            if True:
                                l = 2 * q + (1 if pa else 0)
                                base = l * CHUNK + h * W
                                nc.tensor.matmul(
                                    out=p1q[
                                        pa : pa + RANK,
                                        half * CHUNK : (half + 1) * CHUNK,
                                    ],
                                    lhsT=WA[
                                        half * C : (half + 1) * C,
                                        h * RANK : (h + 1) * RANK,
                                    ],
                                    rhs=X[
                                        half * C : (half + 1) * C,
                                        base : base + CHUNK,
                                    ],
                                    start=(h == 0),
                                    stop=(h == FH - 1),
                                    skip_group_check=True,
                                )
                    t2q = t2_pool.tile([2 * RANK, 2 * CHUNK + 4], dt_c, tag="t2")
                    # pad cols feed only discarded output columns; cheap DVE
                    # memset (gpsimd's Q7 dispatch latency would gate stage B)
                    nc.vector.memset(t2q[:, 2 * CHUNK : 2 * CHUNK + 4], 0.0)
                    if "copies" in ABLATE:
                        nc.vector.memset(t2q[:, 0:8], 0.0)
                    else:
                        # split the evac across both engines so neither
                        # becomes the A->B critical path
                        nc.vector.tensor_copy(
                            out=t2q[:, 0:CHUNK], in_=p1q[:, 0:CHUNK]
                        )
                        nc.scalar.copy(
                            out=t2q[:, CHUNK : 2 * CHUNK],
                            in_=p1q[:, CHUNK : 2 * CHUNK],
                        )
                    return t2q

                yf = y.rearrange("f r w -> f (r w)")
                y4 = y.rearrange("f (g r) w -> f g r w", g=2)
                pair_ot = [None]

                def stage_b(q, t2q):
                    # t2q[pa:pa+64, half*512+w : ...] holds chunk (half, 2q+pa/64)
                    # -> p2q cols (pa/64)*512 -> y rows half*16+4q..+4.
                    for half in range(2):
                        p2q = p2_pool.tile([F, 2 * CHUNK], dt_f32, tag="p2")
                        if "stage_b" in ABLATE:
                            nc.vector.memset(p2q[:, 0:8], 0.0)
                        else:
                            for w in range(FW):
                                for pa in (0, 64):
                                    nc.tensor.matmul(
                                        out=p2q[
                                            :,
                                            (pa // 64) * CHUNK : (pa // 64 + 1)
                                            * CHUNK,
                                        ],
                                        lhsT=WB[
                                            pa : pa + RANK, w * F : (w + 1) * F
                                        ],
                                        rhs=t2q[
                                            pa : pa + RANK,
                                            half * CHUNK
                                            + w : (half + 1) * CHUNK
                                            + w,
                                        ],
                                        start=(w == 0),
                                        stop=(w == FW - 1),
                                        skip_group_check=True,
                                    )
                        if "out_dma" in ABLATE:
                            nc.vector.memset(p2q[:, 8:16], 1.0)
                            continue
                        if "pairstore" in FEATURES and q < nquad - 1:
                            # pair both halves' 4 rows into one 8-row store
                            # (strided across the two half row-blocks)
                            if half == 0:
                                pair_ot[0] = ot_pool.tile(
                                    [F, 4 * CHUNK], dt_c, tag="otp",
                                    name=f"otp{q}",
                                )
                            otq = pair_ot[0]
                            dst = otq[:, half * 2 * CHUNK : (half + 1) * 2 * CHUNK]
                            if (q + half) % 2 == 0:
                                nc.vector.tensor_copy(out=dst, in_=p2q[:])
                            else:
                                nc.scalar.copy(out=dst, in_=p2q[:])
                            if half == 1:
                                ring = nc.sync if q % 2 == 0 else nc.scalar
                                ring.dma_start(
                                    out=y4[:, :, 4 * q : 4 * q + 4, :],
                                    in_=otq[:],
                                )
                            continue
                        ot = ot_pool.tile([F, 2 * CHUNK], dt_c, tag=f"ot{half}")
                        r0 = half * HALF_OUT + 4 * q
                        if q == nquad - 1 and "finalsplit" in FEATURES:
                            # final quad: split evac across both engines and
                            # store 2-row pieces -> short tail
                            nc.vector.tensor_copy(
                                out=ot[:, 0:CHUNK], in_=p2q[:, 0:CHUNK]
                            )
                            nc.scalar.copy(
                                out=ot[:, CHUNK : 2 * CHUNK],
                                in_=p2q[:, CHUNK : 2 * CHUNK],
                            )
                            nc.sync.dma_start(
                                out=yf[:, r0 * W : (r0 + 2) * W],
                                in_=ot[:, 0:CHUNK],
                            )
                            nc.scalar.dma_start(
                                out=yf[:, (r0 + 2) * W : (r0 + 4) * W],
                                in_=ot[:, CHUNK : 2 * CHUNK],
                            )
                            continue
                        if (q + half) % 2 == 0:
                            nc.vector.tensor_copy(out=ot[:], in_=p2q[:])
                            ring = nc.sync
                        else:
                            nc.scalar.copy(out=ot[:], in_=p2q[:])
                            ring = nc.scalar
                        # 4-row store right behind its evac on the matching
                        # ring so the wait is already satisfied at issue
                        ring.dma_start(
                            out=yf[:, r0 * W : (r0 + 4) * W], in_=ot[:]
                        )

                pending = None
                for q in range(nquad + 1):
                    t2q = stage_a(q) if q < nquad else None
                    if pending is not None:
                        stage_b(pending[0], pending[1])
                    pending = (q, t2q) if t2q is not None else None

            if reps == 1:
                body()
            else:
                with tc.For_i(0, reps, 1):
                    body()
            if bench_internal:
                nc.sync.dma_start(out=tout[:], in_=tin[:])

    nc.compile()
    return nc


def _get_program():
    key = (ROWS, COMPUTE_DT)
    if key not in _PROGRAM_CACHE:
        _PROGRAM_CACHE[key] = build_program()
    return _PROGRAM_CACHE[key]


def make_weight_inputs(factor0, factor1, factor2, factor3, np_dt=None):
    np_dt = np_dt or _np_compute_dtype()
    f0 = np.asarray(factor0, np.float32)
    f1 = np.asarray(factor1, np.float32)
    f2 = np.asarray(factor2, np.float32)
    f3 = np.asarray(factor3, np.float32)
    # wa[c, h*RANK+r] = f3[c,r] * f1[h,r], duplicated into both halves
    wa = (f3[:, None, :] * f1[None, :, :]).reshape(C, FH * RANK)
    wa2 = np.concatenate([wa, wa], axis=0).astype(np_dt)
    # wb[r, w*F+f] = f2[w,r] * f0[f,r], duplicated into both halves
    wb = (f2.T[:, :, None] * f0.T[:, None, :]).reshape(RANK, FW * F)
    wb2 = np.concatenate([wb, wb], axis=0).astype(np_dt)
    return np.ascontiguousarray(wa2), np.ascontiguousarray(wb2)


ROW_STARTS = [0, 32, 64, 96, 128, 160, 192, 222]


def kernel(input, factor0, factor1, factor2, factor3):
    from concourse.bass_utils import run_bass_kernel_spmd

    nc = _get_program()
    np_dt = _np_compute_dtype()
    wa2, wb2 = make_weight_inputs(factor0, factor1, factor2, factor3, np_dt)
    inp = np.asarray(input, np.float32).astype(np_dt)
    if "wab" in FEATURES:
        wmap = {"wab": np.ascontiguousarray(np.concatenate([wa2, wb2], axis=1))}
    else:
        wmap = {"wa2": wa2, "wb2": wb2}
    in_maps = []
    for s in ROW_STARTS:
        xs = inp[:, s : s + IN_ROWS, :]
        # partitions (g c): half0 rows 0-17, half1 rows 16-33 -> [2C, 18*W]
        xd = np.stack(
            [xs[:, 0:HALF_IN, :], xs[:, HALF_OUT:IN_ROWS, :]], axis=0
        ).reshape(2 * C, HCOLS)
        in_maps.append({"x": np.ascontiguousarray(xd), **wmap})
    res = run_bass_kernel_spmd(nc, in_maps, list(range(NCORES))).results
    out = np.empty((F, HO, WO), np.float32)
    for i, s in enumerate(ROW_STARTS):
        ys = res[i]["y"][:, :, 0:WO].astype(np.float32)
        if i < NCORES - 1:
            out[:, s : s + ROWS, :] = ys
        else:
            out[:, 224:HO, :] = ys[:, 2:ROWS, :]
    return out


# revision 22
# speedup vs baseline: 1.1211x; 1.0163x over previous
"""CP-decomposed 3x3 conv on 8 TRN2 NeuronCores.

Math: out[f,i,j] = sum_{h,w,c,r} in[c,i+h,j+w] * f1[h,r] * f2[w,r] * f3[c,r] * f0[f,r]

Factorization used on-device (per core, over its 32 output rows):
  stage A: t2[r, n]  = sum_h sum_c (f3[c,r]*f1[h,r]) * x[c, n + h*W]     (3 matmuls, K=C)
  stage B: out[f, n] = sum_w sum_r (f2[w,r]*f0[f,r]) * t2[r, n + w]      (3 matmuls, K=R)
where n flattens (row, col) with row pitch W=256; output cols 254/255 of each
row are garbage and are dropped at host gather.

Per-core layout (v4): the 32 output rows split into two 16-row halves. SBUF
partitions 0-63 hold half0's input rows [0,18), partitions 64-127 hold half1's
rows [16,34). The host ships x with the 2 halo rows duplicated ([C, 36, W]:
rows 0-17 then 16-33) so a single rearranged DMA covers all 128 partitions.

The whole input lives in one SBUF tile [128, 4608], loaded in 4 column-chunk
DMAs ordered by when compute needs them: sync ring carries cols [0,1536) and
[1536,2560); scalar carries the weights then cols [2560,3584) and [3584,4608).
Range-based Tile deps let stage A's first matmuls start after the first chunk.

Per quad-iter q (4 output rows per half): stage A runs 12 matmuls (3 taps x 4
PE quadrants via tile_position auto-derived from lhsT/psum base partitions)
into p1q [128,1024]; the t2 evacuation is split across vector+scalar so both
engines carry half. Stage B runs per half: 6 matmuls (3 taps x 2 row-group
chunks, alternating row groups so fill/drain pipelines) into p2q [128,1024],
evacuated [128,1024] f32->bf16 on alternating engines, each evac immediately
followed by its 4-row y DMA on the ring matching the evac engine (vector->
sync, scalar->scalar) so a waiting DMA never head-blocks the copy engine.

I/O is bf16 both ways; output rows are written 256-wide, 4 rows per DMA, and
trimmed to 254 at host gather.

Sharding: output rows (Ho=254) split across 8 cores: cores 0-6 get rows
[32i, 32i+32); core 7 processes rows [222, 254) via a shifted window (its
first 2 rows duplicate core 6's tail and are dropped at gather).
"""

import sys

sys.path.insert(0, "/opt/trn_rl_repo")

import numpy as np

# Problem constants (hardcoded per contract)
C = 64
H = 256
W = 256
FH = 3
FW = 3
RANK = 64
F = 128
HO = H - FH + 1  # 254
WO = W - FW + 1  # 254
NCORES = 8
ROWS = 32  # output rows per core
IN_ROWS = ROWS + 2  # 34
HALF_OUT = ROWS // 2  # 16 output rows per half
HALF_IN = HALF_OUT + 2  # 18 input rows per half
HCOLS = HALF_IN * W  # 4608 input cols per half
CHUNK = 512  # output elements per chunk (= 2 rows x 256)
NQUAD = 4  # quad-iters; each covers 2 chunks per half (4 rows per half)

COMPUTE_DT = "bf16"
# Ablation switches for benchmarking: subset of
# {"in_dma", "out_dma", "stage_a", "stage_b", "copies"}
ABLATE = set()
# feature flags for A/B benching. Adopted: t2buf3 (deeper t2 staging
# rotation) and ot4 (deeper output staging so evacuations never wait on
# store completion of the backlogged DMA device). Everything else measured
# neutral-to-worse on HW; see memory/trn2-axon-hw-facts.md.
FEATURES = {"t2buf3", "ot4"}

_PROGRAM_CACHE = {}


def _np_compute_dtype():
    import ml_dtypes

    if COMPUTE_DT == "fp16":
        return np.dtype(ml_dtypes.float16)
    return np.dtype(ml_dtypes.bfloat16)


def build_program(
    rows=ROWS,
    compute_dt=None,
    num_devices=NCORES,
    reps=1,
    paired=None,  # unused; kept for bench.py compat
    bench_internal=False,
    nquad=NQUAD,  # bench-only: fewer quad-iters to measure pipeline scaling
):
    """Build + compile the per-core Bass program."""
    from concourse import bacc, mybir, tile

    compute_dt = compute_dt or COMPUTE_DT
    dt_c = mybir.dt.float16 if compute_dt == "fp16" else mybir.dt.bfloat16
    dt_f32 = mybir.dt.float32

    assert rows == ROWS

    nc = bacc.Bacc(
        "TRN2", target_bir_lowering=False, debug=False, num_devices=num_devices
    )
    if bench_internal:
        x = nc.dram_tensor("x_int", [2 * C, HCOLS], dt_c).ap()
        wab = nc.dram_tensor("wab_int", [2 * C, FH * RANK + FW * F], dt_c).ap()
        wa2 = nc.dram_tensor("wa2_int", [2 * C, FH * RANK], dt_c).ap()
        wb2 = nc.dram_tensor("wb2_int", [2 * RANK, FW * F], dt_c).ap()
        y = nc.dram_tensor("y_int", [F, ROWS, W], dt_c).ap()
        tin = nc.dram_tensor("tin", [1, 16], dt_f32, kind="ExternalInput").ap()
        tout = nc.dram_tensor("tout", [1, 16], dt_f32, kind="ExternalOutput").ap()
    else:
        x = nc.dram_tensor("x", [2 * C, HCOLS], dt_c, kind="ExternalInput").ap()
        if "wab" in FEATURES:
            wab = nc.dram_tensor(
                "wab", [2 * C, FH * RANK + FW * F], dt_c, kind="ExternalInput"
            ).ap()
        else:
            wa2 = nc.dram_tensor("wa2", [2 * C, FH * RANK], dt_c, kind="ExternalInput").ap()
            wb2 = nc.dram_tensor("wb2", [2 * RANK, FW * F], dt_c, kind="ExternalInput").ap()
        y = nc.dram_tensor("y", [F, ROWS, W], dt_c, kind="ExternalOutput").ap()

    with tile.TileContext(nc) as tc:
        with (
            tc.tile_pool(name="xin", bufs=2) as xin_pool,
            tc.tile_pool(name="wgt", bufs=2) as wgt_pool,
            tc.tile_pool(name="t2", bufs=(3 if "t2buf3" in FEATURES else 2)) as t2_pool,
            tc.tile_pool(
                name="ot", bufs=(4 if "ot4" in FEATURES else 2)
            ) as ot_pool,
            tc.tile_pool(
                name="p1",
                bufs=(1 if "p2deep" in FEATURES else 2),
                space="PSUM",
            ) as p1_pool,
            tc.tile_pool(
                name="p2",
                bufs=(3 if "p2deep" in FEATURES else 2),
                space="PSUM",
            ) as p2_pool,
        ):

            def body():
                # (g c) partition layout: partitions 0-63 = half0 rows 0-17,
                # 64-127 = half1 rows 16-33 (host duplicates the halo rows).
                X = xin_pool.tile([2 * C, HCOLS], dt_c, tag="x")
                if "wab" in FEATURES:
                    # both weights packed into one DMA on the scalar ring
                    WAB = wgt_pool.tile(
                        [2 * C, FH * RANK + FW * F], dt_c, tag="wab"
                    )
                    nc.scalar.dma_start(out=WAB[:], in_=wab[:])
                    WA = WAB[:, 0 : FH * RANK]
                    WB = WAB[:, FH * RANK : FH * RANK + FW * F]
                else:
                    WA = wgt_pool.tile([2 * C, FH * RANK], dt_c, tag="wa")
                    WB = wgt_pool.tile([2 * RANK, FW * F], dt_c, tag="wb")
                    nc.scalar.dma_start(out=WA[:], in_=wa2[:])
                    nc.scalar.dma_start(out=WB[:], in_=wb2[:])
                if "in_dma" in ABLATE:
                    nc.vector.memset(X[:, 0:8], 0.0)
                else:
                    # ordered by need: q0/q1 cols on sync, q2/q3 on
                    # scalar (rdsync: ALL reads on sync so the next rep's
                    # loads never queue behind evac-gated stores)
                    xmax = nquad * 1024 + 512
                    late = nc.sync if "rdsync" in FEATURES else nc.scalar
                    nc.sync.dma_start(out=X[:, 0:1536], in_=x[:, 0:1536])
                    if xmax > 1536:
                        nc.sync.dma_start(
                            out=X[:, 1536:min(xmax, 2560)],
                            in_=x[:, 1536:min(xmax, 2560)],
                        )
                    if xmax > 2560:
                        late.dma_start(
                            out=X[:, 2560:min(xmax, 3584)],
                            in_=x[:, 2560:min(xmax, 3584)],
                        )
                    if xmax > 3584:
                        late.dma_start(
                            out=X[:, 3584:4608], in_=x[:, 3584:4608]
                        )

                def stage_a(q):
                    # psum slots (pa, col ca): pa = chunk parity, ca = half.
                    # Quadrant (64*half, pa): all four distinct -> 4-way.
                    p1q = p1_pool.tile([2 * C, 2 * CHUNK], dt_f32)
                    if "stage_a" in ABLATE:
                        nc.vector.memset(p1q[:, 0:8], 0.0)
                    else:
                        if q == 0 and "q0phase" in FEATURES:
                            # parity-phased: par0 chains need only x cols
                            # [0,1024) (first DMA chunk), par1 cols [512,1536)
                            order = [
                                (h, half, pa)
                                for pa in (0, 64)
                                for h in range(FH)
                                for half in (0, 1)
                            ]
                        else:
                            order = [
                                (h, half, pa)
                                for h in range(FH)
                                for half, pa in ((0, 0), (0, 64), (1, 0), (1, 64))
                            ]
                        for h, half, pa in order:
                            if True:
                                l = 2 * q + (1 if pa else 0)
                                base = l * CHUNK + h * W
                                nc.tensor.matmul(
                                    out=p1q[
                                        pa : pa + RANK,
                                        half * CHUNK : (half + 1) * CHUNK,
                                    ],
                                    lhsT=WA[
                                        half * C : (half + 1) * C,
                                        h * RANK : (h + 1) * RANK,
                                    ],
                                    rhs=X[
                                        half * C : (half + 1) * C,
                                        base : base + CHUNK,
                                    ],
                                    start=(h == 0),
                                    stop=(h == FH - 1),
                                    skip_group_check=True,
                                )
                    t2q = t2_pool.tile([2 * RANK, 2 * CHUNK + 4], dt_c, tag="t2")
                    # pad cols feed only discarded output columns; cheap DVE
                    # memset (gpsimd's Q7 dispatch latency would gate stage B)
                    nc.vector.memset(t2q[:, 2 * CHUNK : 2 * CHUNK + 4], 0.0)
                    if "copies" in ABLATE:
                        nc.vector.memset(t2q[:, 0:8], 0.0)
                    else:
                        # split the evac across both engines so neither
                        # becomes the A->B critical path
                        nc.vector.tensor_copy(
                            out=t2q[:, 0:CHUNK], in_=p1q[:, 0:CHUNK]
                        )
                        nc.scalar.copy(
                            out=t2q[:, CHUNK : 2 * CHUNK],
                            in_=p1q[:, CHUNK : 2 * CHUNK],
                        )
                    return t2q

                yf = y.rearrange("f r w -> f (r w)")
                y4 = y.rearrange("f (g r) w -> f g r w", g=2)
                pair_ot = [None]
                pend_store = []

                def stage_b(q, t2q):
                    # t2q[pa:pa+64, half*512+w : ...] holds chunk (half, 2q+pa/64)
                    # -> p2q cols (pa/64)*512 -> y rows half*16+4q..+4.
                    for half in range(2):
                        p2q = p2_pool.tile([F, 2 * CHUNK], dt_f32, tag="p2")
                        if "stage_b" in ABLATE:
                            nc.vector.memset(p2q[:, 0:8], 0.0)
                        else:
                            for w in range(FW):
                                for pa in (0, 64):
                                    nc.tensor.matmul(
                                        out=p2q[
                                            :,
                                            (pa // 64) * CHUNK : (pa // 64 + 1)
                                            * CHUNK,
                                        ],
                                        lhsT=WB[
                                            pa : pa + RANK, w * F : (w + 1) * F
                                        ],
                                        rhs=t2q[
                                            pa : pa + RANK,
                                            half * CHUNK
                                            + w : (half + 1) * CHUNK
                                            + w,
                                        ],
                                        start=(w == 0),
                                        stop=(w == FW - 1),
                                        skip_group_check=True,
                                    )
                        if "out_dma" in ABLATE:
                            nc.vector.memset(p2q[:, 8:16], 1.0)
                            continue
                        if "pairstore" in FEATURES and q < nquad - 1:
                            # pair both halves' 4 rows into one 8-row store
                            # (strided across the two half row-blocks)
                            if half == 0:
                                pair_ot[0] = ot_pool.tile(
                                    [F, 4 * CHUNK], dt_c, tag="otp",
                                    name=f"otp{q}",
                                )
                            otq = pair_ot[0]
                            dst = otq[:, half * 2 * CHUNK : (half + 1) * 2 * CHUNK]
                            if (q + half) % 2 == 0:
                                nc.vector.tensor_copy(out=dst, in_=p2q[:])
                            else:
                                nc.scalar.copy(out=dst, in_=p2q[:])
                            if half == 1:
                                ring = nc.sync if q % 2 == 0 else nc.scalar
                                ring.dma_start(
                                    out=y4[:, :, 4 * q : 4 * q + 4, :],
                                    in_=otq[:],
                                )
                            continue
                        ot = ot_pool.tile([F, 2 * CHUNK], dt_c, tag=f"ot{half}")
                        r0 = half * HALF_OUT + 4 * q
                        if "rdsync" in FEATURES:
                            if (q + half) % 2 == 0:
                                nc.vector.tensor_copy(out=ot[:], in_=p2q[:])
                            else:
                                nc.scalar.copy(out=ot[:], in_=p2q[:])
                            pend_store.append(
                                (yf[:, r0 * W : (r0 + 4) * W], ot)
                            )
                            if half == 1:
                                # both stores issue on scalar once both evacs
                                # are in flight; waits are satisfied in order
                                for dst, src_ in pend_store:
                                    nc.scalar.dma_start(out=dst, in_=src_[:])
                                pend_store.clear()
                            continue
                        if q == nquad - 1 and "finalsplit" in FEATURES:
                            # final quad: split evac across both engines and
                            # store 2-row pieces -> short tail
                            nc.vector.tensor_copy(
                                out=ot[:, 0:CHUNK], in_=p2q[:, 0:CHUNK]
                            )
                            nc.scalar.copy(
                                out=ot[:, CHUNK : 2 * CHUNK],
                                in_=p2q[:, CHUNK : 2 * CHUNK],
                            )
                            nc.sync.dma_start(
                                out=yf[:, r0 * W : (r0 + 2) * W],
                                in_=ot[:, 0:CHUNK],
                            )
                            nc.scalar.dma_start(
                                out=yf[:, (r0 + 2) * W : (r0 + 4) * W],
                                in_=ot[:, CHUNK : 2 * CHUNK],
                            )
                            continue
                        if (q + half) % 2 == 0:
                            nc.vector.tensor_copy(out=ot[:], in_=p2q[:])
                            ring = nc.sync
                        else:
                            nc.scalar.copy(out=ot[:], in_=p2q[:])
                            ring = nc.scalar
                        # 4-row store right behind its evac on the matching
                        # ring so the wait is already satisfied at issue
                        ring.dma_start(
                            out=yf[:, r0 * W : (r0 + 4) * W], in_=ot[:]
                        )

                pending = None
                for q in range(nquad + 1):
                    t2q = stage_a(q) if q < nquad else None
                    if pending is not None:
                        stage_b(pending[0], pending[1])
                    pending = (q, t2q) if t2q is not None else None

            if reps == 1:
                body()
            else:
                with tc.For_i(0, reps, 1):
                    body()
            if bench_internal:
                nc.sync.dma_start(out=tout[:], in_=tin[:])

    nc.compile()
    return nc


def _get_program():
    key = (ROWS, COMPUTE_DT)
    if key not in _PROGRAM_CACHE:
        _PROGRAM_CACHE[key] = build_program()
    return _PROGRAM_CACHE[key]


def make_weight_inputs(factor0, factor1, factor2, factor3, np_dt=None):
    np_dt = np_dt or _np_compute_dtype()
    f0 = np.asarray(factor0, np.float32)
    f1 = np.asarray(factor1, np.float32)
    f2 = np.asarray(factor2, np.float32)
    f3 = np.asarray(factor3, np.float32)
    # wa[c, h*RANK+r] = f3[c,r] * f1[h,r], duplicated into both halves
    wa = (f3[:, None, :] * f1[None, :, :]).reshape(C, FH * RANK)
    wa2 = np.concatenate([wa, wa], axis=0).astype(np_dt)
    # wb[r, w*F+f] = f2[w,r] * f0[f,r], duplicated into both halves
    wb = (f2.T[:, :, None] * f0.T[:, None, :]).reshape(RANK, FW * F)
    wb2 = np.concatenate([wb, wb], axis=0).astype(np_dt)
    return np.ascontiguousarray(wa2), np.ascontiguousarray(wb2)


ROW_STARTS = [0, 32, 64, 96, 128, 160, 192, 222]


def kernel(input, factor0, factor1, factor2, factor3):
    from concourse.bass_utils import run_bass_kernel_spmd

    nc = _get_program()
    np_dt = _np_compute_dtype()
    wa2, wb2 = make_weight_inputs(factor0, factor1, factor2, factor3, np_dt)
    inp = np.asarray(input, np.float32).astype(np_dt)
    if "wab" in FEATURES:
        wmap = {"wab": np.ascontiguousarray(np.concatenate([wa2, wb2], axis=1))}
    else:
        wmap = {"wa2": wa2, "wb2": wb2}
    in_maps = []
    for s in ROW_STARTS:
        xs = inp[:, s : s + IN_ROWS, :]
        # partitions (g c): half0 rows 0-17, half1 rows 16-33 -> [2C, 18*W]
        xd = np.stack(
            [xs[:, 0:HALF_IN, :], xs[:, HALF_OUT:IN_ROWS, :]], axis=0
        ).reshape(2 * C, HCOLS)
        in_maps.append({"x": np.ascontiguousarray(xd), **wmap})
    res = run_bass_kernel_spmd(nc, in_maps, list(range(NCORES))).results
    out = np.empty((F, HO, WO), np.float32)
    for i, s in enumerate(ROW_STARTS):
        ys = res[i]["y"][:, :, 0:WO].astype(np.float32)
        if i < NCORES - 1:
            out[:, s : s + ROWS, :] = ys
        else:
            out[:, 224:HO, :] = ys[:, 2:ROWS, :]
    return out


# revision 24
# speedup vs baseline: 1.1449x; 1.0212x over previous
"""CP-decomposed 3x3 conv on 8 TRN2 NeuronCores.

Math: out[f,i,j] = sum_{h,w,c,r} in[c,i+h,j+w] * f1[h,r] * f2[w,r] * f3[c,r] * f0[f,r]

Factorization used on-device (per core, over its 32 output rows):
  stage A: t2[r, n]  = sum_h sum_c (f3[c,r]*f1[h,r]) * x[c, n + h*W]     (3 matmuls, K=C)
  stage B: out[f, n] = sum_w sum_r (f2[w,r]*f0[f,r]) * t2[r, n + w]      (3 matmuls, K=R)
where n flattens (row, col) with row pitch W=256; output cols 254/255 of each
row are garbage and are dropped at host gather.

Per-core layout (v4): the 32 output rows split into two 16-row halves. SBUF
partitions 0-63 hold half0's input rows [0,18), partitions 64-127 hold half1's
rows [16,34). The host ships x with the 2 halo rows duplicated ([C, 36, W]:
rows 0-17 then 16-33) so a single rearranged DMA covers all 128 partitions.

The whole input lives in one SBUF tile [128, 4608], loaded in 4 column-chunk
DMAs ordered by when compute needs them: sync ring carries cols [0,1536) and
[1536,2560); scalar carries the weights then cols [2560,3584) and [3584,4608).
Range-based Tile deps let stage A's first matmuls start after the first chunk.

Per quad-iter q (4 output rows per half): stage A runs 12 matmuls (3 taps x 4
PE quadrants via tile_position auto-derived from lhsT/psum base partitions)
into p1q [128,1024]; the t2 evacuation is split across vector+scalar so both
engines carry half. Stage B runs per half: 6 matmuls (3 taps x 2 row-group
chunks, alternating row groups so fill/drain pipelines) into p2q [128,1024],
evacuated [128,1024] f32->bf16 on alternating engines, each evac immediately
followed by its 4-row y DMA on the ring matching the evac engine (vector->
sync, scalar->scalar) so a waiting DMA never head-blocks the copy engine.

I/O is bf16 both ways; output rows are written 256-wide, 4 rows per DMA, and
trimmed to 254 at host gather.

Sharding: output rows (Ho=254) split across 8 cores: cores 0-6 get rows
[32i, 32i+32); core 7 processes rows [222, 254) via a shifted window (its
first 2 rows duplicate core 6's tail and are dropped at gather).
"""

import sys

sys.path.insert(0, "/opt/trn_rl_repo")

import numpy as np

# Problem constants (hardcoded per contract)
C = 64
H = 256
W = 256
FH = 3
FW = 3
RANK = 64
F = 128
HO = H - FH + 1  # 254
WO = W - FW + 1  # 254
NCORES = 8
ROWS = 32  # output rows per core
IN_ROWS = ROWS + 2  # 34
HALF_OUT = ROWS // 2  # 16 output rows per half
HALF_IN = HALF_OUT + 2  # 18 input rows per half
HCOLS = HALF_IN * W  # 4608 input cols per half
CHUNK = 512  # output elements per chunk (= 2 rows x 256)
NQUAD = 4  # quad-iters; each covers 2 chunks per half (4 rows per half)

COMPUTE_DT = "bf16"
# Ablation switches for benchmarking: subset of
# {"in_dma", "out_dma", "stage_a", "stage_b", "copies"}
ABLATE = set()
# feature flags for A/B benching. Adopted: t2buf3 (deeper t2 staging
# rotation) and ot4 (deeper output staging so evacuations never wait on
# store completion of the backlogged DMA device). Everything else measured
# neutral-to-worse on HW; see memory/trn2-axon-hw-facts.md.
FEATURES = {"t2buf3", "ot4", "stsync"}

_PROGRAM_CACHE = {}


def _np_compute_dtype():
    import ml_dtypes

    if COMPUTE_DT == "fp16":
        return np.dtype(ml_dtypes.float16)
    return np.dtype(ml_dtypes.bfloat16)


def build_program(
    rows=ROWS,
    compute_dt=None,
    num_devices=NCORES,
    reps=1,
    paired=None,  # unused; kept for bench.py compat
    bench_internal=False,
    nquad=NQUAD,  # bench-only: fewer quad-iters to measure pipeline scaling
):
    """Build + compile the per-core Bass program."""
    from concourse import bacc, mybir, tile

    compute_dt = compute_dt or COMPUTE_DT
    dt_c = mybir.dt.float16 if compute_dt == "fp16" else mybir.dt.bfloat16
    dt_f32 = mybir.dt.float32

    assert rows == ROWS

    nc = bacc.Bacc(
        "TRN2", target_bir_lowering=False, debug=False, num_devices=num_devices
    )
    if bench_internal:
        x = nc.dram_tensor("x_int", [2 * C, HCOLS], dt_c).ap()
        wab = nc.dram_tensor("wab_int", [2 * C, FH * RANK + FW * F], dt_c).ap()
        wa2 = nc.dram_tensor("wa2_int", [2 * C, FH * RANK], dt_c).ap()
        wb2 = nc.dram_tensor("wb2_int", [2 * RANK, FW * F], dt_c).ap()
        y = nc.dram_tensor("y_int", [F, ROWS, W], dt_c).ap()
        tin = nc.dram_tensor("tin", [1, 16], dt_f32, kind="ExternalInput").ap()
        tout = nc.dram_tensor("tout", [1, 16], dt_f32, kind="ExternalOutput").ap()
    else:
        x = nc.dram_tensor("x", [2 * C, HCOLS], dt_c, kind="ExternalInput").ap()
        if "wab" in FEATURES:
            wab = nc.dram_tensor(
                "wab", [2 * C, FH * RANK + FW * F], dt_c, kind="ExternalInput"
            ).ap()
        else:
            wa2 = nc.dram_tensor("wa2", [2 * C, FH * RANK], dt_c, kind="ExternalInput").ap()
            wb2 = nc.dram_tensor("wb2", [2 * RANK, FW * F], dt_c, kind="ExternalInput").ap()
        y = nc.dram_tensor("y", [F, ROWS, W], dt_c, kind="ExternalOutput").ap()

    with tile.TileContext(nc) as tc:
        with (
            tc.tile_pool(name="xin", bufs=2) as xin_pool,
            tc.tile_pool(name="wgt", bufs=2) as wgt_pool,
            tc.tile_pool(name="t2", bufs=(3 if "t2buf3" in FEATURES else 2)) as t2_pool,
            tc.tile_pool(
                name="ot", bufs=(4 if "ot4" in FEATURES else 2)
            ) as ot_pool,
            tc.tile_pool(
                name="p1",
                bufs=(1 if "p2deep" in FEATURES else 2),
                space="PSUM",
            ) as p1_pool,
            tc.tile_pool(
                name="p2",
                bufs=(3 if "p2deep" in FEATURES else 2),
                space="PSUM",
            ) as p2_pool,
        ):

            def body():
                # (g c) partition layout: partitions 0-63 = half0 rows 0-17,
                # 64-127 = half1 rows 16-33 (host duplicates the halo rows).
                X = xin_pool.tile([2 * C, HCOLS], dt_c, tag="x")
                if "wab" in FEATURES:
                    # both weights packed into one DMA on the scalar ring
                    WAB = wgt_pool.tile(
                        [2 * C, FH * RANK + FW * F], dt_c, tag="wab"
                    )
                    nc.scalar.dma_start(out=WAB[:], in_=wab[:])
                    WA = WAB[:, 0 : FH * RANK]
                    WB = WAB[:, FH * RANK : FH * RANK + FW * F]
                else:
                    WA = wgt_pool.tile([2 * C, FH * RANK], dt_c, tag="wa")
                    WB = wgt_pool.tile([2 * RANK, FW * F], dt_c, tag="wb")
                    nc.scalar.dma_start(out=WA[:], in_=wa2[:])
                    nc.scalar.dma_start(out=WB[:], in_=wb2[:])
                if "in_dma" in ABLATE:
                    nc.vector.memset(X[:, 0:8], 0.0)
                else:
                    # ordered by need: q0/q1 cols on sync, q2/q3 on
                    # scalar (rdsync: ALL reads on sync so the next rep's
                    # loads never queue behind evac-gated stores)
                    xmax = nquad * 1024 + 512
                    late = nc.sync if "rdsync" in FEATURES else nc.scalar
                    nc.sync.dma_start(out=X[:, 0:1536], in_=x[:, 0:1536])
                    if xmax > 1536:
                        nc.sync.dma_start(
                            out=X[:, 1536:min(xmax, 2560)],
                            in_=x[:, 1536:min(xmax, 2560)],
                        )
                    if xmax > 2560:
                        late.dma_start(
                            out=X[:, 2560:min(xmax, 3584)],
                            in_=x[:, 2560:min(xmax, 3584)],
                        )
                    if xmax > 3584:
                        late.dma_start(
                            out=X[:, 3584:4608], in_=x[:, 3584:4608]
                        )

                def stage_a(q):
                    # psum slots (pa, col ca): pa = chunk parity, ca = half.
                    # Quadrant (64*half, pa): all four distinct -> 4-way.
                    p1q = p1_pool.tile([2 * C, 2 * CHUNK], dt_f32)
                    if "stage_a" in ABLATE:
                        nc.vector.memset(p1q[:, 0:8], 0.0)
                    else:
                        if q == 0 and "q0phase" in FEATURES:
                            # parity-phased: par0 chains need only x cols
                            # [0,1024) (first DMA chunk), par1 cols [512,1536)
                            order = [
                                (h, half, pa)
                                for pa in (0, 64)
                                for h in range(FH)
                                for half in (0, 1)
                            ]
                        else:
                            order = [
                                (h, half, pa)
                                for h in range(FH)
                                for half, pa in ((0, 0), (0, 64), (1, 0), (1, 64))
                            ]
                        for h, half, pa in order:
                            if True:
                                l = 2 * q + (1 if pa else 0)
                                base = l * CHUNK + h * W
                                nc.tensor.matmul(
                                    out=p1q[
                                        pa : pa + RANK,
                                        half * CHUNK : (half + 1) * CHUNK,
                                    ],
                                    lhsT=WA[
                                        half * C : (half + 1) * C,
                                        h * RANK : (h + 1) * RANK,
                                    ],
                                    rhs=X[
                                        half * C : (half + 1) * C,
                                        base : base + CHUNK,
                                    ],
                                    start=(h == 0),
                                    stop=(h == FH - 1),
                                    skip_group_check=True,
                                )
                    t2q = t2_pool.tile([2 * RANK, 2 * CHUNK + 4], dt_c, tag="t2")
                    # pad cols feed only discarded output columns; cheap DVE
                    # memset (gpsimd's Q7 dispatch latency would gate stage B)
                    nc.vector.memset(t2q[:, 2 * CHUNK : 2 * CHUNK + 4], 0.0)
                    if "copies" in ABLATE:
                        nc.vector.memset(t2q[:, 0:8], 0.0)
                    else:
                        # split the evac across both engines so neither
                        # becomes the A->B critical path
                        nc.vector.tensor_copy(
                            out=t2q[:, 0:CHUNK], in_=p1q[:, 0:CHUNK]
                        )
                        nc.scalar.copy(
                            out=t2q[:, CHUNK : 2 * CHUNK],
                            in_=p1q[:, CHUNK : 2 * CHUNK],
                        )
                    return t2q

                yf = y.rearrange("f r w -> f (r w)")
                y4 = y.rearrange("f (g r) w -> f g r w", g=2)
                pair_ot = [None]
                pend_store = []

                def stage_b(q, t2q):
                    # t2q[pa:pa+64, half*512+w : ...] holds chunk (half, 2q+pa/64)
                    # -> p2q cols (pa/64)*512 -> y rows half*16+4q..+4.
                    for half in range(2):
                        p2q = p2_pool.tile([F, 2 * CHUNK], dt_f32, tag="p2")
                        if "stage_b" in ABLATE:
                            nc.vector.memset(p2q[:, 0:8], 0.0)
                        else:
                            for w in range(FW):
                                for pa in (0, 64):
                                    nc.tensor.matmul(
                                        out=p2q[
                                            :,
                                            (pa // 64) * CHUNK : (pa // 64 + 1)
                                            * CHUNK,
                                        ],
                                        lhsT=WB[
                                            pa : pa + RANK, w * F : (w + 1) * F
                                        ],
                                        rhs=t2q[
                                            pa : pa + RANK,
                                            half * CHUNK
                                            + w : (half + 1) * CHUNK
                                            + w,
                                        ],
                                        start=(w == 0),
                                        stop=(w == FW - 1),
                                        skip_group_check=True,
                                    )
                        if "out_dma" in ABLATE:
                            nc.vector.memset(p2q[:, 8:16], 1.0)
                            continue
                        if "pairstore" in FEATURES and q < nquad - 1:
                            # pair both halves' 4 rows into one 8-row store
                            # (strided across the two half row-blocks)
                            if half == 0:
                                pair_ot[0] = ot_pool.tile(
                                    [F, 4 * CHUNK], dt_c, tag="otp",
                                    name=f"otp{q}",
                                )
                            otq = pair_ot[0]
                            dst = otq[:, half * 2 * CHUNK : (half + 1) * 2 * CHUNK]
                            if (q + half) % 2 == 0:
                                nc.vector.tensor_copy(out=dst, in_=p2q[:])
                            else:
                                nc.scalar.copy(out=dst, in_=p2q[:])
                            if half == 1:
                                ring = nc.sync if q % 2 == 0 else nc.scalar
                                ring.dma_start(
                                    out=y4[:, :, 4 * q : 4 * q + 4, :],
                                    in_=otq[:],
                                )
                            continue
                        ot = ot_pool.tile([F, 2 * CHUNK], dt_c, tag=f"ot{half}")
                        r0 = half * HALF_OUT + 4 * q
                        if "rdsync" in FEATURES:
                            if (q + half) % 2 == 0:
                                nc.vector.tensor_copy(out=ot[:], in_=p2q[:])
                            else:
                                nc.scalar.copy(out=ot[:], in_=p2q[:])
                            pend_store.append(
                                (yf[:, r0 * W : (r0 + 4) * W], ot)
                            )
                            if half == 1:
                                # both stores issue on scalar once both evacs
                                # are in flight; waits are satisfied in order
                                for dst, src_ in pend_store:
                                    nc.scalar.dma_start(out=dst, in_=src_[:])
                                pend_store.clear()
                            continue
                        if q == nquad - 1 and "finalsplit" in FEATURES:
                            # final quad: split evac across both engines and
                            # store 2-row pieces -> short tail
                            nc.vector.tensor_copy(
                                out=ot[:, 0:CHUNK], in_=p2q[:, 0:CHUNK]
                            )
                            nc.scalar.copy(
                                out=ot[:, CHUNK : 2 * CHUNK],
                                in_=p2q[:, CHUNK : 2 * CHUNK],
                            )
                            nc.sync.dma_start(
                                out=yf[:, r0 * W : (r0 + 2) * W],
                                in_=ot[:, 0:CHUNK],
                            )
                            nc.scalar.dma_start(
                                out=yf[:, (r0 + 2) * W : (r0 + 4) * W],
                                in_=ot[:, CHUNK : 2 * CHUNK],
                            )
                            continue
                        if (q + half) % 2 == 0:
                            nc.vector.tensor_copy(out=ot[:], in_=p2q[:])
                            ring = nc.sync
                        else:
                            nc.scalar.copy(out=ot[:], in_=p2q[:])
                            # stsync: ACT never issues DMAs, so its SEQ can
                            # never block on HWDGE arbitration under backlog
                            ring = nc.sync if "stsync" in FEATURES else nc.scalar
                        # 4-row store right behind its evac on the matching
                        # ring so the wait is already satisfied at issue
                        ring.dma_start(
                            out=yf[:, r0 * W : (r0 + 4) * W], in_=ot[:]
                        )

                pending = None
                for q in range(nquad + 1):
                    t2q = stage_a(q) if q < nquad else None
                    if pending is not None:
                        stage_b(pending[0], pending[1])
                    pending = (q, t2q) if t2q is not None else None

            if reps == 1:
                body()
            else:
                with tc.For_i(0, reps, 1):
                    body()
            if bench_internal:
                nc.sync.dma_start(out=tout[:], in_=tin[:])

    nc.compile()
    return nc


def _get_program():
    key = (ROWS, COMPUTE_DT)
    if key not in _PROGRAM_CACHE:
        _PROGRAM_CACHE[key] = build_program()
    return _PROGRAM_CACHE[key]


def make_weight_inputs(factor0, factor1, factor2, factor3, np_dt=None):
    np_dt = np_dt or _np_compute_dtype()
    f0 = np.asarray(factor0, np.float32)
    f1 = np.asarray(factor1, np.float32)
    f2 = np.asarray(factor2, np.float32)
    f3 = np.asarray(factor3, np.float32)
    # wa[c, h*RANK+r] = f3[c,r] * f1[h,r], duplicated into both halves
    wa = (f3[:, None, :] * f1[None, :, :]).reshape(C, FH * RANK)
    wa2 = np.concatenate([wa, wa], axis=0).astype(np_dt)
    # wb[r, w*F+f] = f2[w,r] * f0[f,r], duplicated into both halves
    wb = (f2.T[:, :, None] * f0.T[:, None, :]).reshape(RANK, FW * F)
    wb2 = np.concatenate([wb, wb], axis=0).astype(np_dt)
    return np.ascontiguousarray(wa2), np.ascontiguousarray(wb2)


ROW_STARTS = [0, 32, 64, 96, 128, 160, 192, 222]


def kernel(input, factor0, factor1, factor2, factor3):
    from concourse.bass_utils import run_bass_kernel_spmd

    nc = _get_program()
    np_dt = _np_compute_dtype()
    wa2, wb2 = make_weight_inputs(factor0, factor1, factor2, factor3, np_dt)
    inp = np.asarray(input, np.float32).astype(np_dt)
    if "wab" in FEATURES:
        wmap = {"wab": np.ascontiguousarray(np.concatenate([wa2, wb2], axis=1))}
    else:
        wmap = {"wa2": wa2, "wb2": wb2}
    in_maps = []
    for s in ROW_STARTS:
        xs = inp[:, s : s + IN_ROWS, :]
        # partitions (g c): half0 rows 0-17, half1 rows 16-33 -> [2C, 18*W]
        xd = np.stack(
            [xs[:, 0:HALF_IN, :], xs[:, HALF_OUT:IN_ROWS, :]], axis=0
        ).reshape(2 * C, HCOLS)
        in_maps.append({"x": np.ascontiguousarray(xd), **wmap})
    res = run_bass_kernel_spmd(nc, in_maps, list(range(NCORES))).results
    out = np.empty((F, HO, WO), np.float32)
    for i, s in enumerate(ROW_STARTS):
        ys = res[i]["y"][:, :, 0:WO].astype(np.float32)
        if i < NCORES - 1:
            out[:, s : s + ROWS, :] = ys
        else:
            out[:, 224:HO, :] = ys[:, 2:ROWS, :]
    return out


# revision 25
# speedup vs baseline: 1.1595x; 1.0128x over previous
"""CP-decomposed 3x3 conv on 8 TRN2 NeuronCores.

Math: out[f,i,j] = sum_{h,w,c,r} in[c,i+h,j+w] * f1[h,r] * f2[w,r] * f3[c,r] * f0[f,r]

Factorization used on-device (per core, over its 32 output rows):
  stage A: t2[r, n]  = sum_h sum_c (f3[c,r]*f1[h,r]) * x[c, n + h*W]     (3 matmuls, K=C)
  stage B: out[f, n] = sum_w sum_r (f2[w,r]*f0[f,r]) * t2[r, n + w]      (3 matmuls, K=R)
where n flattens (row, col) with row pitch W=256; output cols 254/255 of each
row are garbage and are dropped at host gather.

Per-core layout (v4): the 32 output rows split into two 16-row halves. SBUF
partitions 0-63 hold half0's input rows [0,18), partitions 64-127 hold half1's
rows [16,34). The host ships x with the 2 halo rows duplicated ([C, 36, W]:
rows 0-17 then 16-33) so a single rearranged DMA covers all 128 partitions.

The whole input lives in one SBUF tile [128, 4608], loaded in 4 column-chunk
DMAs ordered by when compute needs them: sync ring carries cols [0,1536) and
[1536,2560); scalar carries the weights then cols [2560,3584) and [3584,4608).
Range-based Tile deps let stage A's first matmuls start after the first chunk.

Per quad-iter q (4 output rows per half): stage A runs 12 matmuls (3 taps x 4
PE quadrants via tile_position auto-derived from lhsT/psum base partitions)
into p1q [128,1024]; the t2 evacuation is split across vector+scalar so both
engines carry half. Stage B runs per half: 6 matmuls (3 taps x 2 row-group
chunks, alternating row groups so fill/drain pipelines) into p2q [128,1024],
evacuated [128,1024] f32->bf16 on alternating engines, each evac immediately
followed by its 4-row y DMA on the ring matching the evac engine (vector->
sync, scalar->scalar) so a waiting DMA never head-blocks the copy engine.

I/O is bf16 both ways; output rows are written 256-wide, 4 rows per DMA, and
trimmed to 254 at host gather.

Sharding: output rows (Ho=254) split across 8 cores: cores 0-6 get rows
[32i, 32i+32); core 7 processes rows [222, 254) via a shifted window (its
first 2 rows duplicate core 6's tail and are dropped at gather).
"""

import sys

sys.path.insert(0, "/opt/trn_rl_repo")

import numpy as np

# Problem constants (hardcoded per contract)
C = 64
H = 256
W = 256
FH = 3
FW = 3
RANK = 64
F = 128
HO = H - FH + 1  # 254
WO = W - FW + 1  # 254
NCORES = 8
ROWS = 32  # output rows per core
IN_ROWS = ROWS + 2  # 34
HALF_OUT = ROWS // 2  # 16 output rows per half
HALF_IN = HALF_OUT + 2  # 18 input rows per half
HCOLS = HALF_IN * W  # 4608 input cols per half
CHUNK = 512  # output elements per chunk (= 2 rows x 256)
NQUAD = 4  # quad-iters; each covers 2 chunks per half (4 rows per half)

COMPUTE_DT = "bf16"
# Ablation switches for benchmarking: subset of
# {"in_dma", "out_dma", "stage_a", "stage_b", "copies"}
ABLATE = set()
# feature flags for A/B benching. Adopted: t2buf3 (deeper t2 staging
# rotation) and ot4 (deeper output staging so evacuations never wait on
# store completion of the backlogged DMA device). Everything else measured
# neutral-to-worse on HW; see memory/trn2-axon-hw-facts.md.
FEATURES = {"t2buf3", "ot4", "stsync"}

_PROGRAM_CACHE = {}


def _np_compute_dtype():
    import ml_dtypes

    if COMPUTE_DT == "fp16":
        return np.dtype(ml_dtypes.float16)
    return np.dtype(ml_dtypes.bfloat16)


def build_program(
    rows=ROWS,
    compute_dt=None,
    num_devices=NCORES,
    reps=1,
    paired=None,  # unused; kept for bench.py compat
    bench_internal=False,
    nquad=NQUAD,  # bench-only: fewer quad-iters to measure pipeline scaling
):
    """Build + compile the per-core Bass program."""
    from concourse import bacc, mybir, tile

    compute_dt = compute_dt or COMPUTE_DT
    dt_c = mybir.dt.float16 if compute_dt == "fp16" else mybir.dt.bfloat16
    dt_f32 = mybir.dt.float32

    assert rows == ROWS

    nc = bacc.Bacc(
        "TRN2", target_bir_lowering=False, debug=False, num_devices=num_devices
    )
    if bench_internal:
        x = nc.dram_tensor("x_int", [2 * C, HCOLS], dt_c).ap()
        wab = nc.dram_tensor("wab_int", [2 * C, FH * RANK + FW * F], dt_c).ap()
        wa2 = nc.dram_tensor("wa2_int", [2 * C, FH * RANK], dt_c).ap()
        wb2 = nc.dram_tensor("wb2_int", [2 * RANK, FW * F], dt_c).ap()
        y = nc.dram_tensor("y_int", [F, ROWS, W], dt_c).ap()
        tin = nc.dram_tensor("tin", [1, 16], dt_f32, kind="ExternalInput").ap()
        tout = nc.dram_tensor("tout", [1, 16], dt_f32, kind="ExternalOutput").ap()
    else:
        x = nc.dram_tensor("x", [2 * C, HCOLS], dt_c, kind="ExternalInput").ap()
        if "wab" in FEATURES:
            wab = nc.dram_tensor(
                "wab", [2 * C, FH * RANK + FW * F], dt_c, kind="ExternalInput"
            ).ap()
        else:
            wa2 = nc.dram_tensor("wa2", [2 * C, FH * RANK], dt_c, kind="ExternalInput").ap()
            wb2 = nc.dram_tensor("wb2", [2 * RANK, FW * F], dt_c, kind="ExternalInput").ap()
        y = nc.dram_tensor("y", [F, ROWS, W], dt_c, kind="ExternalOutput").ap()

    with tile.TileContext(nc) as tc:
        with (
            tc.tile_pool(name="xin", bufs=2) as xin_pool,
            tc.tile_pool(name="wgt", bufs=2) as wgt_pool,
            tc.tile_pool(name="t2", bufs=(3 if "t2buf3" in FEATURES else 2)) as t2_pool,
            tc.tile_pool(
                name="ot", bufs=(4 if "ot4" in FEATURES else 2)
            ) as ot_pool,
            tc.tile_pool(
                name="p1",
                bufs=(1 if "p2deep" in FEATURES else 2),
                space="PSUM",
            ) as p1_pool,
            tc.tile_pool(
                name="p2",
                bufs=(3 if "p2deep" in FEATURES else 2),
                space="PSUM",
            ) as p2_pool,
        ):

            def body():
                # (g c) partition layout: partitions 0-63 = half0 rows 0-17,
                # 64-127 = half1 rows 16-33 (host duplicates the halo rows).
                X = xin_pool.tile([2 * C, HCOLS], dt_c, tag="x")
                if "wab" in FEATURES:
                    # both weights packed into one DMA on the scalar ring
                    WAB = wgt_pool.tile(
                        [2 * C, FH * RANK + FW * F], dt_c, tag="wab"
                    )
                    nc.scalar.dma_start(out=WAB[:], in_=wab[:])
                    WA = WAB[:, 0 : FH * RANK]
                    WB = WAB[:, FH * RANK : FH * RANK + FW * F]
                else:
                    WA = wgt_pool.tile([2 * C, FH * RANK], dt_c, tag="wa")
                    WB = wgt_pool.tile([2 * RANK, FW * F], dt_c, tag="wb")
                    if "actfree" in FEATURES:
                        # scalar engine issues NO DMAs at all: X1 leads sync
                        # (stage A's critical input), weights right behind
                        if "in_dma" not in ABLATE:
                            nc.sync.dma_start(
                                out=X[:, 0:1536], in_=x[:, 0:1536]
                            )
                        nc.sync.dma_start(out=WA[:], in_=wa2[:])
                        nc.sync.dma_start(out=WB[:], in_=wb2[:])
                    else:
                        nc.scalar.dma_start(out=WA[:], in_=wa2[:])
                        nc.scalar.dma_start(out=WB[:], in_=wb2[:])
                if "in_dma" in ABLATE:
                    nc.vector.memset(X[:, 0:8], 0.0)
                else:
                    # ordered by need: q0/q1 cols on sync, q2/q3 on
                    # scalar (rdsync: ALL reads on sync so the next rep's
                    # loads never queue behind evac-gated stores)
                    xmax = nquad * 1024 + 512
                    late = (
                        nc.sync
                        if ("rdsync" in FEATURES or "actfree" in FEATURES)
                        else nc.scalar
                    )
                    if "actfree" not in FEATURES:
                        nc.sync.dma_start(out=X[:, 0:1536], in_=x[:, 0:1536])
                    if xmax > 1536:
                        nc.sync.dma_start(
                            out=X[:, 1536:min(xmax, 2560)],
                            in_=x[:, 1536:min(xmax, 2560)],
                        )
                    if xmax > 2560:
                        late.dma_start(
                            out=X[:, 2560:min(xmax, 3584)],
                            in_=x[:, 2560:min(xmax, 3584)],
                        )
                    if xmax > 3584:
                        late.dma_start(
                            out=X[:, 3584:4608], in_=x[:, 3584:4608]
                        )

                def stage_a(q):
                    # psum slots (pa, col ca): pa = chunk parity, ca = half.
                    # Quadrant (64*half, pa): all four distinct -> 4-way.
                    p1q = p1_pool.tile([2 * C, 2 * CHUNK], dt_f32)
                    if "stage_a" in ABLATE:
                        nc.vector.memset(p1q[:, 0:8], 0.0)
                    else:
                        if q == 0 and "q0phase" in FEATURES:
                            # parity-phased: par0 chains need only x cols
                            # [0,1024) (first DMA chunk), par1 cols [512,1536)
                            order = [
                                (h, half, pa)
                                for pa in (0, 64)
                                for h in range(FH)
                                for half in (0, 1)
                            ]
                        else:
                            order = [
                                (h, half, pa)
                                for h in range(FH)
                                for half, pa in ((0, 0), (0, 64), (1, 0), (1, 64))
                            ]
                        for h, half, pa in order:
                            if True:
                                l = 2 * q + (1 if pa else 0)
                                base = l * CHUNK + h * W
                                nc.tensor.matmul(
                                    out=p1q[
                                        pa : pa + RANK,
                                        half * CHUNK : (half + 1) * CHUNK,
                                    ],
                                    lhsT=WA[
                                        half * C : (half + 1) * C,
                                        h * RANK : (h + 1) * RANK,
                                    ],
                                    rhs=X[
                                        half * C : (half + 1) * C,
                                        base : base + CHUNK,
                                    ],
                                    start=(h == 0),
                                    stop=(h == FH - 1),
                                    skip_group_check=True,
                                )
                    t2q = t2_pool.tile([2 * RANK, 2 * CHUNK + 4], dt_c, tag="t2")
                    # pad cols feed only discarded output columns; cheap DVE
                    # memset (gpsimd's Q7 dispatch latency would gate stage B)
                    nc.vector.memset(t2q[:, 2 * CHUNK : 2 * CHUNK + 4], 0.0)
                    if "copies" in ABLATE:
                        nc.vector.memset(t2q[:, 0:8], 0.0)
                    else:
                        # split the evac across both engines so neither
                        # becomes the A->B critical path
                        nc.vector.tensor_copy(
                            out=t2q[:, 0:CHUNK], in_=p1q[:, 0:CHUNK]
                        )
                        nc.scalar.copy(
                            out=t2q[:, CHUNK : 2 * CHUNK],
                            in_=p1q[:, CHUNK : 2 * CHUNK],
                        )
                    return t2q

                yf = y.rearrange("f r w -> f (r w)")
                y4 = y.rearrange("f (g r) w -> f g r w", g=2)
                pair_ot = [None]
                pend_store = []

                def stage_b(q, t2q):
                    # t2q[pa:pa+64, half*512+w : ...] holds chunk (half, 2q+pa/64)
                    # -> p2q cols (pa/64)*512 -> y rows half*16+4q..+4.
                    for half in range(2):
                        p2q = p2_pool.tile([F, 2 * CHUNK], dt_f32, tag="p2")
                        if "stage_b" in ABLATE:
                            nc.vector.memset(p2q[:, 0:8], 0.0)
                        else:
                            for w in range(FW):
                                for pa in (0, 64):
                                    nc.tensor.matmul(
                                        out=p2q[
                                            :,
                                            (pa // 64) * CHUNK : (pa // 64 + 1)
                                            * CHUNK,
                                        ],
                                        lhsT=WB[
                                            pa : pa + RANK, w * F : (w + 1) * F
                                        ],
                                        rhs=t2q[
                                            pa : pa + RANK,
                                            half * CHUNK
                                            + w : (half + 1) * CHUNK
                                            + w,
                                        ],
                                        start=(w == 0),
                                        stop=(w == FW - 1),
                                        skip_group_check=True,
                                    )
                        if "out_dma" in ABLATE:
                            nc.vector.memset(p2q[:, 8:16], 1.0)
                            continue
                        if "pairstore" in FEATURES and q < nquad - 1:
                            # pair both halves' 4 rows into one 8-row store
                            # (strided across the two half row-blocks)
                            if half == 0:
                                pair_ot[0] = ot_pool.tile(
                                    [F, 4 * CHUNK], dt_c, tag="otp",
                                    name=f"otp{q}",
                                )
                            otq = pair_ot[0]
                            dst = otq[:, half * 2 * CHUNK : (half + 1) * 2 * CHUNK]
                            if (q + half) % 2 == 0:
                                nc.vector.tensor_copy(out=dst, in_=p2q[:])
                            else:
                                nc.scalar.copy(out=dst, in_=p2q[:])
                            if half == 1:
                                ring = nc.sync if q % 2 == 0 else nc.scalar
                                ring.dma_start(
                                    out=y4[:, :, 4 * q : 4 * q + 4, :],
                                    in_=otq[:],
                                )
                            continue
                        ot = ot_pool.tile([F, 2 * CHUNK], dt_c, tag=f"ot{half}")
                        r0 = half * HALF_OUT + 4 * q
                        if "rdsync" in FEATURES:
                            if (q + half) % 2 == 0:
                                nc.vector.tensor_copy(out=ot[:], in_=p2q[:])
                            else:
                                nc.scalar.copy(out=ot[:], in_=p2q[:])
                            pend_store.append(
                                (yf[:, r0 * W : (r0 + 4) * W], ot)
                            )
                            if half == 1:
                                # both stores issue on scalar once both evacs
                                # are in flight; waits are satisfied in order
                                for dst, src_ in pend_store:
                                    nc.scalar.dma_start(out=dst, in_=src_[:])
                                pend_store.clear()
                            continue
                        if q == nquad - 1 and "finalsplit" in FEATURES:
                            # final quad: split evac across both engines and
                            # store 2-row pieces -> short tail
                            nc.vector.tensor_copy(
                                out=ot[:, 0:CHUNK], in_=p2q[:, 0:CHUNK]
                            )
                            nc.scalar.copy(
                                out=ot[:, CHUNK : 2 * CHUNK],
                                in_=p2q[:, CHUNK : 2 * CHUNK],
                            )
                            nc.sync.dma_start(
                                out=yf[:, r0 * W : (r0 + 2) * W],
                                in_=ot[:, 0:CHUNK],
                            )
                            nc.scalar.dma_start(
                                out=yf[:, (r0 + 2) * W : (r0 + 4) * W],
                                in_=ot[:, CHUNK : 2 * CHUNK],
                            )
                            continue
                        if (q + half) % 2 == 0:
                            nc.vector.tensor_copy(out=ot[:], in_=p2q[:])
                            ring = nc.sync
                        else:
                            nc.scalar.copy(out=ot[:], in_=p2q[:])
                            # stsync: ACT never issues DMAs, so its SEQ can
                            # never block on HWDGE arbitration under backlog
                            ring = nc.sync if "stsync" in FEATURES else nc.scalar
                        # 4-row store right behind its evac on the matching
                        # ring so the wait is already satisfied at issue
                        ring.dma_start(
                            out=yf[:, r0 * W : (r0 + 4) * W], in_=ot[:]
                        )

                pending = None
                for q in range(nquad + 1):
                    t2q = stage_a(q) if q < nquad else None
                    if pending is not None:
                        stage_b(pending[0], pending[1])
                    pending = (q, t2q) if t2q is not None else None

            if reps == 1:
                body()
            else:
                with tc.For_i(0, reps, 1):
                    body()
            if bench_internal:
                nc.sync.dma_start(out=tout[:], in_=tin[:])

    nc.compile()
    return nc


def _get_program():
    key = (ROWS, COMPUTE_DT)
    if key not in _PROGRAM_CACHE:
        _PROGRAM_CACHE[key] = build_program()
    return _PROGRAM_CACHE[key]


def make_weight_inputs(factor0, factor1, factor2, factor3, np_dt=None):
    np_dt = np_dt or _np_compute_dtype()
    f0 = np.asarray(factor0, np.float32)
    f1 = np.asarray(factor1, np.float32)
    f2 = np.asarray(factor2, np.float32)
    f3 = np.asarray(factor3, np.float32)
    # wa[c, h*RANK+r] = f3[c,r] * f1[h,r], duplicated into both halves
    wa = (f3[:, None, :] * f1[None, :, :]).reshape(C, FH * RANK)
    wa2 = np.concatenate([wa, wa], axis=0).astype(np_dt)
    # wb[r, w*F+f] = f2[w,r] * f0[f,r], duplicated into both halves
    wb = (f2.T[:, :, None] * f0.T[:, None, :]).reshape(RANK, FW * F)
    wb2 = np.concatenate([wb, wb], axis=0).astype(np_dt)
    return np.ascontiguousarray(wa2), np.ascontiguousarray(wb2)


ROW_STARTS = [0, 32, 64, 96, 128, 160, 192, 222]


def kernel(input, factor0, factor1, factor2, factor3):
    from concourse.bass_utils import run_bass_kernel_spmd

    nc = _get_program()
    np_dt = _np_compute_dtype()
    wa2, wb2 = make_weight_inputs(factor0, factor1, factor2, factor3, np_dt)
    inp = np.asarray(input, np.float32).astype(np_dt)
    if "wab" in FEATURES:
        wmap = {"wab": np.ascontiguousarray(np.concatenate([wa2, wb2], axis=1))}
    else:
        wmap = {"wa2": wa2, "wb2": wb2}
    in_maps = []
    for s in ROW_STARTS:
        xs = inp[:, s : s + IN_ROWS, :]
        # partitions (g c): half0 rows 0-17, half1 rows 16-33 -> [2C, 18*W]
        xd = np.stack(
            [xs[:, 0:HALF_IN, :], xs[:, HALF_OUT:IN_ROWS, :]], axis=0
        ).reshape(2 * C, HCOLS)
        in_maps.append({"x": np.ascontiguousarray(xd), **wmap})
    res = run_bass_kernel_spmd(nc, in_maps, list(range(NCORES))).results
    out = np.empty((F, HO, WO), np.float32)
    for i, s in enumerate(ROW_STARTS):
        ys = res[i]["y"][:, :, 0:WO].astype(np.float32)
        if i < NCORES - 1:
            out[:, s : s + ROWS, :] = ys
        else:
            out[:, 224:HO, :] = ys[:, 2:ROWS, :]
    return out


# revision 26
# speedup vs baseline: 1.1652x; 1.0049x over previous
"""CP-decomposed 3x3 conv on 8 TRN2 NeuronCores.

Math: out[f,i,j] = sum_{h,w,c,r} in[c,i+h,j+w] * f1[h,r] * f2[w,r] * f3[c,r] * f0[f,r]

Factorization used on-device (per core, over its 32 output rows):
  stage A: t2[r, n]  = sum_h sum_c (f3[c,r]*f1[h,r]) * x[c, n + h*W]     (3 matmuls, K=C)
  stage B: out[f, n] = sum_w sum_r (f2[w,r]*f0[f,r]) * t2[r, n + w]      (3 matmuls, K=R)
where n flattens (row, col) with row pitch W=256; output cols 254/255 of each
row are garbage and are dropped at host gather.

Per-core layout (v4): the 32 output rows split into two 16-row halves. SBUF
partitions 0-63 hold half0's input rows [0,18), partitions 64-127 hold half1's
rows [16,34). The host ships x with the 2 halo rows duplicated ([C, 36, W]:
rows 0-17 then 16-33) so a single rearranged DMA covers all 128 partitions.

The whole input lives in one SBUF tile [128, 4608], loaded in 4 column-chunk
DMAs ordered by when compute needs them: sync ring carries cols [0,1536) and
[1536,2560); scalar carries the weights then cols [2560,3584) and [3584,4608).
Range-based Tile deps let stage A's first matmuls start after the first chunk.

Per quad-iter q (4 output rows per half): stage A runs 12 matmuls (3 taps x 4
PE quadrants via tile_position auto-derived from lhsT/psum base partitions)
into p1q [128,1024]; the t2 evacuation is split across vector+scalar so both
engines carry half. Stage B runs per half: 6 matmuls (3 taps x 2 row-group
chunks, alternating row groups so fill/drain pipelines) into p2q [128,1024],
evacuated [128,1024] f32->bf16 on alternating engines, each evac immediately
followed by its 4-row y DMA on the ring matching the evac engine (vector->
sync, scalar->scalar) so a waiting DMA never head-blocks the copy engine.

I/O is bf16 both ways; output rows are written 256-wide, 4 rows per DMA, and
trimmed to 254 at host gather.

Sharding: output rows (Ho=254) split across 8 cores: cores 0-6 get rows
[32i, 32i+32); core 7 processes rows [222, 254) via a shifted window (its
first 2 rows duplicate core 6's tail and are dropped at gather).
"""

import sys

sys.path.insert(0, "/opt/trn_rl_repo")

import numpy as np

# Problem constants (hardcoded per contract)
C = 64
H = 256
W = 256
FH = 3
FW = 3
RANK = 64
F = 128
HO = H - FH + 1  # 254
WO = W - FW + 1  # 254
NCORES = 8
ROWS = 32  # output rows per core
IN_ROWS = ROWS + 2  # 34
HALF_OUT = ROWS // 2  # 16 output rows per half
HALF_IN = HALF_OUT + 2  # 18 input rows per half
HCOLS = HALF_IN * W  # 4608 input cols per half
CHUNK = 512  # output elements per chunk (= 2 rows x 256)
NQUAD = 4  # quad-iters; each covers 2 chunks per half (4 rows per half)

COMPUTE_DT = "bf16"
# Ablation switches for benchmarking: subset of
# {"in_dma", "out_dma", "stage_a", "stage_b", "copies"}
ABLATE = set()
# feature flags for A/B benching. Adopted: t2buf3 (deeper t2 staging
# rotation) and ot4 (deeper output staging so evacuations never wait on
# store completion of the backlogged DMA device). Everything else measured
# neutral-to-worse on HW; see memory/trn2-axon-hw-facts.md.
FEATURES = {"t2buf3", "ot4", "stsync"}

_PROGRAM_CACHE = {}


def _np_compute_dtype():
    import ml_dtypes

    if COMPUTE_DT == "fp16":
        return np.dtype(ml_dtypes.float16)
    return np.dtype(ml_dtypes.bfloat16)


def build_program(
    rows=ROWS,
    compute_dt=None,
    num_devices=NCORES,
    reps=1,
    paired=None,  # unused; kept for bench.py compat
    bench_internal=False,
    nquad=NQUAD,  # bench-only: fewer quad-iters to measure pipeline scaling
):
    """Build + compile the per-core Bass program."""
    from concourse import bacc, mybir, tile

    compute_dt = compute_dt or COMPUTE_DT
    dt_c = mybir.dt.float16 if compute_dt == "fp16" else mybir.dt.bfloat16
    dt_f32 = mybir.dt.float32

    assert rows == ROWS

    nc = bacc.Bacc(
        "TRN2", target_bir_lowering=False, debug=False, num_devices=num_devices
    )
    if bench_internal:
        x = nc.dram_tensor("x_int", [2 * C, HCOLS], dt_c).ap()
        wab = nc.dram_tensor("wab_int", [2 * C, FH * RANK + FW * F], dt_c).ap()
        wa2 = nc.dram_tensor("wa2_int", [2 * C, FH * RANK], dt_c).ap()
        wb2 = nc.dram_tensor("wb2_int", [2 * RANK, FW * F], dt_c).ap()
        y = nc.dram_tensor("y_int", [F, ROWS, W], dt_c).ap()
        tin = nc.dram_tensor("tin", [1, 16], dt_f32, kind="ExternalInput").ap()
        tout = nc.dram_tensor("tout", [1, 16], dt_f32, kind="ExternalOutput").ap()
    else:
        x = nc.dram_tensor("x", [2 * C, HCOLS], dt_c, kind="ExternalInput").ap()
        if "wab" in FEATURES:
            wab = nc.dram_tensor(
                "wab", [2 * C, FH * RANK + FW * F], dt_c, kind="ExternalInput"
            ).ap()
        else:
            wa2 = nc.dram_tensor("wa2", [2 * C, FH * RANK], dt_c, kind="ExternalInput").ap()
            wb2 = nc.dram_tensor("wb2", [2 * RANK, FW * F], dt_c, kind="ExternalInput").ap()
        y = nc.dram_tensor("y", [F, ROWS, W], dt_c, kind="ExternalOutput").ap()

    with tile.TileContext(nc) as tc:
        with (
            tc.tile_pool(name="xin", bufs=2) as xin_pool,
            tc.tile_pool(name="wgt", bufs=2) as wgt_pool,
            tc.tile_pool(name="t2", bufs=(3 if "t2buf3" in FEATURES else 2)) as t2_pool,
            tc.tile_pool(
                name="ot", bufs=(4 if "ot4" in FEATURES else 2)
            ) as ot_pool,
            tc.tile_pool(
                name="p1",
                bufs=(1 if "p2deep" in FEATURES else 2),
                space="PSUM",
            ) as p1_pool,
            tc.tile_pool(
                name="p2",
                bufs=(3 if "p2deep" in FEATURES else 2),
                space="PSUM",
            ) as p2_pool,
        ):

            def body():
                # (g c) partition layout: partitions 0-63 = half0 rows 0-17,
                # 64-127 = half1 rows 16-33 (host duplicates the halo rows).
                X = xin_pool.tile([2 * C, HCOLS], dt_c, tag="x")
                if "wab" in FEATURES:
                    # both weights packed into one DMA on the scalar ring
                    WAB = wgt_pool.tile(
                        [2 * C, FH * RANK + FW * F], dt_c, tag="wab"
                    )
                    nc.scalar.dma_start(out=WAB[:], in_=wab[:])
                    WA = WAB[:, 0 : FH * RANK]
                    WB = WAB[:, FH * RANK : FH * RANK + FW * F]
                else:
                    WA = wgt_pool.tile([2 * C, FH * RANK], dt_c, tag="wa")
                    WB = wgt_pool.tile([2 * RANK, FW * F], dt_c, tag="wb")
                    if "actfree" in FEATURES:
                        # scalar engine issues NO DMAs at all: X1 leads sync
                        # (stage A's critical input), weights right behind
                        if "in_dma" not in ABLATE:
                            nc.sync.dma_start(
                                out=X[:, 0:1536], in_=x[:, 0:1536]
                            )
                        nc.sync.dma_start(out=WA[:], in_=wa2[:])
                        nc.sync.dma_start(out=WB[:], in_=wb2[:])
                    else:
                        nc.scalar.dma_start(out=WA[:], in_=wa2[:])
                        nc.scalar.dma_start(out=WB[:], in_=wb2[:])
                if "in_dma" in ABLATE:
                    nc.vector.memset(X[:, 0:8], 0.0)
                else:
                    # ordered by need: q0/q1 cols on sync, q2/q3 on
                    # scalar (rdsync: ALL reads on sync so the next rep's
                    # loads never queue behind evac-gated stores)
                    xmax = nquad * 1024 + 512
                    late = (
                        nc.sync
                        if ("rdsync" in FEATURES or "actfree" in FEATURES)
                        else nc.scalar
                    )
                    if "actfree" not in FEATURES:
                        nc.sync.dma_start(out=X[:, 0:1536], in_=x[:, 0:1536])
                    if xmax > 1536:
                        nc.sync.dma_start(
                            out=X[:, 1536:min(xmax, 2560)],
                            in_=x[:, 1536:min(xmax, 2560)],
                        )
                    if xmax > 2560:
                        late.dma_start(
                            out=X[:, 2560:min(xmax, 3584)],
                            in_=x[:, 2560:min(xmax, 3584)],
                        )
                    if xmax > 3584:
                        late.dma_start(
                            out=X[:, 3584:4608], in_=x[:, 3584:4608]
                        )

                def stage_a(q):
                    # psum slots (pa, col ca): pa = chunk parity, ca = half.
                    # Quadrant (64*half, pa): all four distinct -> 4-way.
                    p1q = p1_pool.tile([2 * C, 2 * CHUNK], dt_f32)
                    if "stage_a" in ABLATE:
                        nc.vector.memset(p1q[:, 0:8], 0.0)
                    else:
                        if q == 0 and "q0phase" in FEATURES:
                            # parity-phased: par0 chains need only x cols
                            # [0,1024) (first DMA chunk), par1 cols [512,1536)
                            order = [
                                (h, half, pa)
                                for pa in (0, 64)
                                for h in range(FH)
                                for half in (0, 1)
                            ]
                        else:
                            order = [
                                (h, half, pa)
                                for h in range(FH)
                                for half, pa in ((0, 0), (0, 64), (1, 0), (1, 64))
                            ]
                        for h, half, pa in order:
                            if True:
                                l = 2 * q + (1 if pa else 0)
                                base = l * CHUNK + h * W
                                nc.tensor.matmul(
                                    out=p1q[
                                        pa : pa + RANK,
                                        half * CHUNK : (half + 1) * CHUNK,
                                    ],
                                    lhsT=WA[
                                        half * C : (half + 1) * C,
                                        h * RANK : (h + 1) * RANK,
                                    ],
                                    rhs=X[
                                        half * C : (half + 1) * C,
                                        base : base + CHUNK,
                                    ],
                                    start=(h == 0),
                                    stop=(h == FH - 1),
                                    skip_group_check=True,
                                )
                    t2q = t2_pool.tile([2 * RANK, 2 * CHUNK + 4], dt_c, tag="t2")
                    # pad cols feed only discarded output columns; cheap DVE
                    # memset (gpsimd's Q7 dispatch latency would gate stage B)
                    nc.vector.memset(t2q[:, 2 * CHUNK : 2 * CHUNK + 4], 0.0)
                    if "copies" in ABLATE:
                        nc.vector.memset(t2q[:, 0:8], 0.0)
                    else:
                        # split the evac across both engines so neither
                        # becomes the A->B critical path
                        nc.vector.tensor_copy(
                            out=t2q[:, 0:CHUNK], in_=p1q[:, 0:CHUNK]
                        )
                        nc.scalar.copy(
                            out=t2q[:, CHUNK : 2 * CHUNK],
                            in_=p1q[:, CHUNK : 2 * CHUNK],
                        )
                    return t2q

                yf = y.rearrange("f r w -> f (r w)")
                y4 = y.rearrange("f (g r) w -> f g r w", g=2)
                pair_ot = [None]
                pend_store = []

                def stage_b(q, t2q):
                    # t2q[pa:pa+64, half*512+w : ...] holds chunk (half, 2q+pa/64)
                    # -> p2q cols (pa/64)*512 -> y rows half*16+4q..+4.
                    for half in range(2):
                        p2q = p2_pool.tile([F, 2 * CHUNK], dt_f32, tag="p2")
                        if "stage_b" in ABLATE:
                            nc.vector.memset(p2q[:, 0:8], 0.0)
                        else:
                            for w in range(FW):
                                for pa in (0, 64):
                                    nc.tensor.matmul(
                                        out=p2q[
                                            :,
                                            (pa // 64) * CHUNK : (pa // 64 + 1)
                                            * CHUNK,
                                        ],
                                        lhsT=WB[
                                            pa : pa + RANK, w * F : (w + 1) * F
                                        ],
                                        rhs=t2q[
                                            pa : pa + RANK,
                                            half * CHUNK
                                            + w : (half + 1) * CHUNK
                                            + w,
                                        ],
                                        start=(w == 0),
                                        stop=(w == FW - 1),
                                        skip_group_check=True,
                                    )
                        if "out_dma" in ABLATE:
                            nc.vector.memset(p2q[:, 8:16], 1.0)
                            continue
                        if "pairstore" in FEATURES and q < nquad - 1:
                            # pair both halves' 4 rows into one 8-row store
                            # (strided across the two half row-blocks)
                            if half == 0:
                                pair_ot[0] = ot_pool.tile(
                                    [F, 4 * CHUNK], dt_c, tag="otp",
                                    name=f"otp{q}",
                                )
                            otq = pair_ot[0]
                            dst = otq[:, half * 2 * CHUNK : (half + 1) * 2 * CHUNK]
                            if (q + half) % 2 == 0:
                                nc.vector.tensor_copy(out=dst, in_=p2q[:])
                            else:
                                nc.scalar.copy(out=dst, in_=p2q[:])
                            if half == 1:
                                ring = nc.sync if q % 2 == 0 else nc.scalar
                                ring.dma_start(
                                    out=y4[:, :, 4 * q : 4 * q + 4, :],
                                    in_=otq[:],
                                )
                            continue
                        ot = ot_pool.tile([F, 2 * CHUNK], dt_c, tag=f"ot{half}")
                        r0 = half * HALF_OUT + 4 * q
                        if "rdsync" in FEATURES:
                            if (q + half) % 2 == 0:
                                nc.vector.tensor_copy(out=ot[:], in_=p2q[:])
                            else:
                                nc.scalar.copy(out=ot[:], in_=p2q[:])
                            pend_store.append(
                                (yf[:, r0 * W : (r0 + 4) * W], ot)
                            )
                            if half == 1:
                                # both stores issue on scalar once both evacs
                                # are in flight; waits are satisfied in order
                                for dst, src_ in pend_store:
                                    nc.scalar.dma_start(out=dst, in_=src_[:])
                                pend_store.clear()
                            continue
                        if q == nquad - 1 and "finalsplit" in FEATURES:
                            # final quad: split evac across both engines and
                            # store 2-row pieces -> short tail
                            nc.vector.tensor_copy(
                                out=ot[:, 0:CHUNK], in_=p2q[:, 0:CHUNK]
                            )
                            nc.scalar.copy(
                                out=ot[:, CHUNK : 2 * CHUNK],
                                in_=p2q[:, CHUNK : 2 * CHUNK],
                            )
                            nc.sync.dma_start(
                                out=yf[:, r0 * W : (r0 + 2) * W],
                                in_=ot[:, 0:CHUNK],
                            )
                            nc.scalar.dma_start(
                                out=yf[:, (r0 + 2) * W : (r0 + 4) * W],
                                in_=ot[:, CHUNK : 2 * CHUNK],
                            )
                            continue
                        if "evsplit" in FEATURES:
                            # split each evac across both engines: halves the
                            # latency on the B->evac->psum-reuse chain (DVE's
                            # pa0 half is ready one matmul earlier) and
                            # balances DVE/ACT; store still one 4-row DMA
                            nc.vector.tensor_copy(
                                out=ot[:, 0:CHUNK], in_=p2q[:, 0:CHUNK]
                            )
                            nc.scalar.copy(
                                out=ot[:, CHUNK : 2 * CHUNK],
                                in_=p2q[:, CHUNK : 2 * CHUNK],
                            )
                            ring = nc.sync
                        elif (q + half) % 2 == 0:
                            nc.vector.tensor_copy(out=ot[:], in_=p2q[:])
                            ring = nc.sync
                        else:
                            nc.scalar.copy(out=ot[:], in_=p2q[:])
                            # stsync: ACT never issues DMAs, so its SEQ can
                            # never block on HWDGE arbitration under backlog
                            ring = nc.sync if "stsync" in FEATURES else nc.scalar
                        # 4-row store right behind its evac on the matching
                        # ring so the wait is already satisfied at issue
                        ring.dma_start(
                            out=yf[:, r0 * W : (r0 + 4) * W], in_=ot[:]
                        )

                pending = None
                for q in range(nquad + 1):
                    t2q = stage_a(q) if q < nquad else None
                    if pending is not None:
                        stage_b(pending[0], pending[1])
                    pending = (q, t2q) if t2q is not None else None

            if reps == 1:
                body()
            else:
                with tc.For_i(0, reps, 1):
                    body()
            if bench_internal:
                nc.sync.dma_start(out=tout[:], in_=tin[:])

    nc.compile()
    return nc


def _get_program():
    key = (ROWS, COMPUTE_DT)
    if key not in _PROGRAM_CACHE:
        _PROGRAM_CACHE[key] = build_program()
    return _PROGRAM_CACHE[key]


def make_weight_inputs(factor0, factor1, factor2, factor3, np_dt=None):
    np_dt = np_dt or _np_compute_dtype()
    f0 = np.asarray(factor0, np.float32)
    f1 = np.asarray(factor1, np.float32)
    f2 = np.asarray(factor2, np.float32)
    f3 = np.asarray(factor3, np.float32)
    # wa[c, h*RANK+r] = f3[c,r] * f1[h,r], duplicated into both halves
    wa = (f3[:, None, :] * f1[None, :, :]).reshape(C, FH * RANK)
    wa2 = np.concatenate([wa, wa], axis=0).astype(np_dt)
    # wb[r, w*F+f] = f2[w,r] * f0[f,r], duplicated into both halves
    wb = (f2.T[:, :, None] * f0.T[:, None, :]).reshape(RANK, FW * F)
    wb2 = np.concatenate([wb, wb], axis=0).astype(np_dt)
    return np.ascontiguousarray(wa2), np.ascontiguousarray(wb2)


ROW_STARTS = [0, 32, 64, 96, 128, 160, 192, 222]


def kernel(input, factor0, factor1, factor2, factor3):
    from concourse.bass_utils import run_bass_kernel_spmd

    nc = _get_program()
    np_dt = _np_compute_dtype()
    wa2, wb2 = make_weight_inputs(factor0, factor1, factor2, factor3, np_dt)
    inp = np.asarray(input, np.float32).astype(np_dt)
    if "wab" in FEATURES:
        wmap = {"wab": np.ascontiguousarray(np.concatenate([wa2, wb2], axis=1))}
    else:
        wmap = {"wa2": wa2, "wb2": wb2}
    in_maps = []
    for s in ROW_STARTS:
        xs = inp[:, s : s + IN_ROWS, :]
        # partitions (g c): half0 rows 0-17, half1 rows 16-33 -> [2C, 18*W]
        xd = np.stack(
            [xs[:, 0:HALF_IN, :], xs[:, HALF_OUT:IN_ROWS, :]], axis=0
        ).reshape(2 * C, HCOLS)
        in_maps.append({"x": np.ascontiguousarray(xd), **wmap})
    res = run_bass_kernel_spmd(nc, in_maps, list(range(NCORES))).results
    out = np.empty((F, HO, WO), np.float32)
    for i, s in enumerate(ROW_STARTS):
        ys = res[i]["y"][:, :, 0:WO].astype(np.float32)
        if i < NCORES - 1:
            out[:, s : s + ROWS, :] = ys
        else:
            out[:, 224:HO, :] = ys[:, 2:ROWS, :]
    return out
